# revision 25
# baseline (speedup 1.0000x reference)
"""Trainium2 Bass kernel for nn_CA1AttentionGate.

Computes, for full inputs (B=1, S=8192, H=1024, F=128, K=2):
    temporal = relu(t @ Wt1 + bt1) @ Wt2 + bt2          [K,F]
    mem      = dg_features + temporal                    [K,F]
    qmean    = query.mean(axis=1)                        [1,H]
    score_k  = tanh([mem_k ; qmean] @ Wa1 + ba1) @ Wa2 + ba2
    w_k      = sigmoid(score_k)
    g_k      = mem_k @ Wg + bg                           [K,H]
    row[s]   = (1/K) * sum_k w_k * (g_k . key[s])        [S]
    out      = broadcast(row) -> [1,1,S,S]

Sharding: sequence-parallel over the key/seq axis across 8 cores.  Each
core computes the final gate row for its 1024 key positions and writes
its dense [8192, 1024] column slab of the output.  The slab is written
in fp16 (well within the 2e-2 tolerance; the host upcasts on gather),
halving the dominant output-write traffic.  The only cross-core
quantity is qmean: each core reduces its query shard via PE matmuls
into PSUM and a 4KB AllGather completes the mean (fallback variant
replicates the full query read instead).
"""

import os

import numpy as np

SEQ = 8192
H = 1024
F = 128
K = 2
NCORES = 8
SHARD = SEQ // NCORES  # 1024
NT = SHARD // 128  # 8 key tiles per shard

_PROG_CACHE = {}


def _build(use_collective: bool):
    import concourse.bacc as bacc
    import concourse.bass as bass
    import concourse.tile as tile
    from concourse import mybir
    from concourse.tile_rust import add_dep_helper

    AF = mybir.ActivationFunctionType
    ALU = mybir.AluOpType
    f32 = mybir.dt.float32
    f32r = mybir.dt.float32r
    f16 = mybir.dt.float16

    nc = bacc.Bacc(
        "TRN2",
        target_bir_lowering=False,
        debug=False,
        num_devices=NCORES,
    )

    def din(name, shape, dt=None):
        return nc.dram_tensor(
            name, list(shape), dt or f32, kind="ExternalInput"
        ).ap()

    q_rows = SHARD if use_collective else SEQ
    # f32r: same bits as f32, but 4x faster PE matmuls (plenty of precision
    # for the 2e-2 tolerance)
    qs = din("qs", (q_rows, H), f32r)
    ks = din("ks", (SHARD, H))
    dg = din("dg", (K, F))
    ts = din("ts", (K,))
    Wt1 = din("Wt1", (1, F // 4))
    bt1 = din("bt1", (F // 4,))
    Wt2 = din("Wt2", (F // 4, F))
    bt2 = din("bt2", (F,))
    Wa1 = din("Wa1", (F + H, F))
    ba1 = din("ba1", (F,))
    Wa2 = din("Wa2", (F, 1))
    ba2 = din("ba2", (1,))
    Wg = din("Wg", (F, H), f32r)
    bg = din("bg", (H,))
    # column of 1/SEQ: the qsum partition-reduce matmul yields the scaled
    # mean contribution directly
    scale_col = din("scale_col", (128, 1), f32r)
    out = nc.dram_tensor("out", [SEQ, SHARD], f16, kind="ExternalOutput").ap()

    def bcast(ap, n):
        # replicate a DRAM row across n partitions (stride-0 partition dim)
        return bass.AP(tensor=ap.tensor, offset=ap.offset, ap=[[0, n]] + list(ap.ap))

    def col(ap, n):
        # load a flat [n] DRAM vector as an [n, 1] column
        return bass.AP(tensor=ap.tensor, offset=ap.offset, ap=[[1, n], [n, 1]])

    with tile.TileContext(nc) as tc:
        with (
            tc.tile_pool(name="consts", bufs=1) as cp,
            tc.tile_pool(name="work", bufs=1) as wp,
            tc.tile_pool(name="qstream", bufs=8) as qp,
            tc.tile_pool(name="scratch", bufs=2) as sp,
            tc.tile_pool(name="psum_small", bufs=1, space="PSUM") as pps,
            tc.tile_pool(name="psum_big", bufs=2, space="PSUM") as ppb,
            tc.tile_pool(name="dram", bufs=1, space="DRAM") as dp,
        ):
            # ---- scale column first (feeds the qsum matmuls) ------------
            sc_c = cp.tile([128, 1], f32r, tag="sc")
            nc.sync.dma_start(sc_c, scale_col)
            # tiny temporal-MLP weights ride the (idle) ACT queue so the
            # query stream owns the sync queue from t=0; the g-vector chain
            # (memT -> g matmuls -> gb broadcasts) is ready before keys land
            Wt2_sb = cp.tile([F // 4, F], f32, tag="Wt2")
            nc.scalar.dma_start(Wt2_sb, Wt2)
            dgT_sb = cp.tile([F, K], f32, tag="dgT")
            nc.scalar.dma_start(dgT_sb, dg.rearrange("k f -> f k"))
            tb_sb = cp.tile([F // 4, K], f32, tag="tb")
            nc.scalar.dma_start(tb_sb, bcast(ts, F // 4))
            Wt1T_sb = cp.tile([F // 4, 1], f32, tag="Wt1T")
            nc.scalar.dma_start(Wt1T_sb, col(Wt1, F // 4))
            bt1T_sb = cp.tile([F // 4, 1], f32, tag="bt1T")
            nc.scalar.dma_start(bt1T_sb, col(bt1, F // 4))
            bt2T_sb = cp.tile([F, 1], f32, tag="bt2T")
            nc.scalar.dma_start(bt2T_sb, col(bt2, F))
            # ---- query shard DMAs get the wire next --------------------
            nq = q_rows // 128
            qv = qs.rearrange("(t p) h -> t p h", p=128)
            qtiles = []
            q_insts = []
            for i in range(nq):
                qt = qp.tile([128, H], f32r, tag="qt")
                q_insts.append(nc.sync.dma_start(qt, qv[i]))
                qtiles.append(qt)

            # ---- gate/temporal weights after the query stream ----------
            Wg_sb = cp.tile([F, H], f32r, tag="Wg")
            w_a0 = nc.sync.dma_start(Wg_sb, Wg)
            add_dep_helper(w_a0.ins, q_insts[-1].ins,
                           reason="gate weights after query")
            bg_sb = cp.tile([1, H], f32, tag="bg")
            w_last = nc.sync.dma_start(bg_sb, bg.rearrange("(a h) -> a h", a=1))

            # warm the ACT function tables used late in the critical path
            warm1 = cp.tile([1, 1], f32, tag="warm1")
            nc.scalar.activation(warm1, sc_c[0:1, :], AF.Tanh)
            warm2 = cp.tile([1, 1], f32, tag="warm2")
            nc.scalar.activation(warm2, sc_c[0:1, :], AF.Sigmoid)

            # key shard: interleaved, ktiles[j][p, :] = ks[p*NT + j, :];
            # explicitly ordered after the small-weight block
            kv = ks.rearrange("(p t) h -> p t h", t=NT)
            ktiles = []
            k_insts = []
            for j in range(NT):
                kt = cp.tile([128, H], f32, tag=f"ks{j}")
                ki = nc.sync.dma_start(kt, kv[:, j, :])
                add_dep_helper(ki.ins, w_last.ins,
                               reason="key reads after early weights")
                ktiles.append(kt)
                k_insts.append(ki)

            # ---- scorer weights (needed only post-collective) ----------
            Wa1m_sb = cp.tile([128, 128], f32, tag="Wa1m")
            wb0 = nc.sync.dma_start(Wa1m_sb, Wa1[0:F, :])
            add_dep_helper(wb0.ins, k_insts[-1].ins,
                           reason="scorer weights after key stream")
            # qmean rows of Wa1 re-paired to the interleaved qmT layout:
            # chunk c pairs with rows {128 + i*8 + c}
            Wa1q_sb = cp.tile([128, 8, 128], f32, tag="Wa1q")
            nc.sync.dma_start(
                Wa1q_sb, Wa1[F : F + H, :].rearrange("(i c) f -> i c f", c=8)
            )
            Wa2_sb = cp.tile([F, 1], f32, tag="Wa2")
            nc.sync.dma_start(Wa2_sb, Wa2)
            ba1T_sb = cp.tile([F, 1], f32, tag="ba1T")
            nc.sync.dma_start(ba1T_sb, col(ba1, F))
            ba2b_sb = cp.tile([1, 1], f32, tag="ba2b")
            nc.sync.dma_start(ba2b_sb, bcast(ba2, 1))

            # ---- qsum on PE: psum[0, h] = sum_s q[s, h] / SEQ -----------
            # (accumulating matmuls keep DVE free for the matvec)
            # psum[0, h] = sum_s q[s, h] / SEQ — accumulating f32r matmuls,
            # one pair per query tile as it lands (keeps DVE free)
            qsum_ps = ppb.tile([1, H], f32, tag="big")
            for i in range(nq):
                nc.tensor.matmul(
                    qsum_ps[:, 0:512], lhsT=sc_c, rhs=qtiles[i][:, 0:512],
                    start=(i == 0), stop=(i == nq - 1),
                )
                nc.tensor.matmul(
                    qsum_ps[:, 512:1024], lhsT=sc_c, rhs=qtiles[i][:, 512:1024],
                    start=(i == 0), stop=(i == nq - 1),
                )
            qpart_sb = wp.tile([1, H], f32, tag="qpart")
            nc.scalar.copy(qpart_sb, qsum_ps)
            if use_collective:
                cc_in = dp.tile([1, H], f32, tag="ccin")
                cc_out = dp.tile([NCORES, H], f32, tag="ccout")
                nc.scalar.dma_start(cc_in, qpart_sb)
                nc.gpsimd.collective_compute(
                    "AllGather",
                    ALU.bypass,
                    replica_groups=[list(range(NCORES))],
                    ins=[cc_in.opt()],
                    outs=[cc_out.opt()],
                )

            # ---- temporal MLP -> memT [F, K] ---------------------------
            h1T = wp.tile([F // 4, K], f32, tag="h1T")
            nc.vector.tensor_scalar_mul(h1T, tb_sb, Wt1T_sb)
            nc.vector.tensor_scalar_add(h1T, h1T, bt1T_sb)
            nc.vector.tensor_relu(h1T, h1T)
            tT_ps = pps.tile([F, K], f32, tag="small")
            nc.tensor.matmul(tT_ps, lhsT=Wt2_sb, rhs=h1T, start=True, stop=True)
            memT_sb = wp.tile([F, K], f32, tag="memT")
            nc.scalar.activation(memT_sb, tT_ps, AF.Identity, bias=bt2T_sb, scale=1.0)
            nc.vector.tensor_add(memT_sb, memT_sb, dgT_sb)
            memTr_sb = wp.tile([F, K], f32r, tag="memTr")
            nc.vector.tensor_copy(memTr_sb, memT_sb)

            # ---- gate rows g_k = mem_k @ Wg + bg  [1, H] ---------------
            def g_row(k):
                g_ps = ppb.tile([1, H], f32, tag="big")
                nc.tensor.matmul(g_ps[:, 0:512], lhsT=memTr_sb[:, k : k + 1],
                                 rhs=Wg_sb[:, 0:512], start=True, stop=True)
                nc.tensor.matmul(g_ps[:, 512:1024], lhsT=memTr_sb[:, k : k + 1],
                                 rhs=Wg_sb[:, 512:1024], start=True, stop=True)
                return g_ps

            g0_ps = g_row(0)
            g0_sb = wp.tile([1, H], f32, tag="g0r")
            nc.vector.tensor_add(g0_sb, g0_ps, bg_sb)
            gb0 = wp.tile([128, H], f32, tag="gb0")
            nc.gpsimd.partition_broadcast(gb0[:, :], g0_sb[:, :])
            g1_ps = g_row(1)
            g1_sb = wp.tile([1, H], f32, tag="g1r")
            nc.vector.tensor_add(g1_sb, g1_ps, bg_sb)
            gb1 = wp.tile([128, H], f32, tag="gb1")
            nc.gpsimd.partition_broadcast(gb1[:, :], g1_sb[:, :])

            # ---- matvec: fused mul+reduce on DVE -----------------------
            # rcc[p, j, k] = sum_h g_k[h] * ks[p*NT+j, h]
            # (NB: tensor_tensor_reduce would fuse mul+reduce on DVE but
            # crashes real TRN2 hardware — keep mul + separate reduce.)
            # Split the 16 muls and 16 reductions across DVE/Pool/ACT so no
            # single engine's serial stream exceeds the key-load window.
            rcc = wp.tile([128, NT, K], f32, tag="rcc")
            pool_mul = {(1, 1), (3, 1), (4, 1), (5, 1), (7, 1)}
            dve_red = {(3, 1), (4, 1), (5, 1), (7, 0), (7, 1)}
            last_dve = last_act = None
            for j in range(NT):
                for k, gb in ((0, gb0), (1, gb1)):
                    if (j, k) in pool_mul:
                        prod = sp.tile([128, H], f32, tag="prodp")
                        nc.gpsimd.tensor_mul(prod, ktiles[j], gb)
                    else:
                        prod = sp.tile([128, H], f32, tag="prodv")
                        last_dve = nc.vector.tensor_mul(prod, ktiles[j], gb)
                    if (j, k) in dve_red:
                        last_dve = nc.vector.tensor_reduce(
                            rcc[:, j, k : k + 1], prod,
                            axis=mybir.AxisListType.X, op=ALU.add,
                        )
                    else:
                        junk = sp.tile([128, H], f32, tag="junk")
                        last_act = nc.scalar.activation(
                            junk, prod, AF.Copy,
                            accum_out=rcc[:, j, k : k + 1],
                        )

            if use_collective:
                # gather-result load parked late on the sync ring (must not
                # block the ACT accum stream behind the collective wait)
                qmTd8 = wp.tile([128, NCORES, 8], f32, tag="qmTd8")
                nc.sync.dma_start(
                    qmTd8, cc_out[:, :].rearrange("d (p c) -> p d c", c=8)
                )

            # reshape both anchors at once to an interleaved row:
            # rTi[0, 2*s + k] = r_k[s]   (s = p*NT + j)
            rTi = wp.tile([1, K * SHARD], f32, tag="rTi")
            nc.scalar.dma_start(rTi[:, :], rcc[:, :, :])
            # broadcast to all partitions while the scorer finishes
            rB = wp.tile([128, SHARD, K], f32, tag="rB")
            nc.gpsimd.partition_broadcast(rB[:, :, :], rTi[:, :])

            # ---- post-collective: qmT, scorer, weights -----------------
            # (on Pool — a DVE/ACT placement would park a collective-gated
            # wait in the middle of the in-order matvec streams)
            # qmT[p, c] = qmean[p*8 + c]  (interleaved reshape layout)
            qmT = wp.tile([128, 8], f32, tag="qmT")
            if use_collective:
                # sum gathered partials over d ([p, c, d] view, reduce X);
                # pinned after the matvec so its collective-gated wait cannot
                # stall the in-order DVE stream mid-matvec
                qr = nc.vector.tensor_reduce(
                    qmT, qmTd8[:, :, :].rearrange("p d c -> p c d"),
                    axis=mybir.AxisListType.X, op=ALU.add,
                )
                add_dep_helper(qr.ins, last_dve.ins,
                               reason="qmT reduce after matvec on DVE")
            else:
                nc.scalar.dma_start(qmT, qpart_sb[:, :])
            qmTd = wp.tile([128, 8, K], f32, tag="qmTd")
            nc.gpsimd.tensor_copy(qmTd[:, :, 0:1], qmT[:, :].rearrange("p c -> p c ()"))
            nc.gpsimd.tensor_copy(qmTd[:, :, 1:2], qmT[:, :].rearrange("p c -> p c ()"))
            haT_ps = pps.tile([F, K], f32, tag="haT")
            nc.tensor.matmul(haT_ps, lhsT=Wa1m_sb, rhs=memT_sb,
                             start=True, stop=False)
            for c in range(8):
                nc.tensor.matmul(haT_ps, lhsT=Wa1q_sb[:, c, :],
                                 rhs=qmTd[:, c, :], start=False, stop=(c == 7))
            aT_sb = wp.tile([F, K], f32, tag="aT")
            th = nc.scalar.activation(aT_sb, haT_ps, AF.Tanh, bias=ba1T_sb, scale=1.0)
            add_dep_helper(th.ins, last_act.ins,
                           reason="tanh after matvec accums on ACT")
            scoreT_ps = pps.tile([1, K], f32, tag="small")
            nc.tensor.matmul(scoreT_ps, lhsT=Wa2_sb, rhs=aT_sb, start=True, stop=True)
            wvT_sb = wp.tile([1, K], f32, tag="wvT")
            nc.scalar.activation(wvT_sb, scoreT_ps, AF.Sigmoid, bias=ba2b_sb, scale=1.0)
            nc.scalar.mul(wvT_sb, wvT_sb, 1.0 / K)
            wvb = wp.tile([128, K], f32, tag="wvb")
            nc.gpsimd.partition_broadcast(wvb[:, :], wvT_sb[:, :])

            # ---- combine anchors (128-wide), fp16 output row -----------
            o_tmp = wp.tile([128, SHARD], f32, tag="o_tmp")
            nc.vector.tensor_scalar_mul(o_tmp, rB[:, :, 1], wvb[:, 1:2])
            out_sb = wp.tile([128, SHARD], f16, tag="osb")
            nc.vector.scalar_tensor_tensor(
                out_sb, rB[:, :, 0], wvb[:, 0:1], o_tmp, ALU.mult, ALU.add
            )

            # ---- output: 64 x [128 rows, SHARD cols], all rows = row ---
            outv = out.rearrange("(b p) n -> b p n", p=128)
            for b in range(SEQ // 128):
                nc.sync.dma_start(outv[b], out_sb)

    nc.compile()
    return nc


def _get_prog(use_collective: bool):
    key = bool(use_collective)
    if key not in _PROG_CACHE:
        _PROG_CACHE[key] = _build(key)
    return _PROG_CACHE[key]


def _make_in_maps(inputs, use_collective: bool):
    q = np.ascontiguousarray(np.asarray(inputs["query"], np.float32)[0])  # [S,H]
    k = np.ascontiguousarray(np.asarray(inputs["key"], np.float32)[0])  # [S,H]
    common = {
        "dg": np.ascontiguousarray(np.asarray(inputs["dg_features"], np.float32)),
        "ts": np.ascontiguousarray(np.asarray(inputs["timestamps"], np.float32)),
        "Wt1": np.ascontiguousarray(np.asarray(inputs["Wt1"], np.float32)),
        "bt1": np.ascontiguousarray(np.asarray(inputs["bt1"], np.float32)),
        "Wt2": np.ascontiguousarray(np.asarray(inputs["Wt2"], np.float32)),
        "bt2": np.ascontiguousarray(np.asarray(inputs["bt2"], np.float32)),
        "Wa1": np.ascontiguousarray(np.asarray(inputs["Wa1"], np.float32)),
        "ba1": np.ascontiguousarray(np.asarray(inputs["ba1"], np.float32)),
        "Wa2": np.ascontiguousarray(np.asarray(inputs["Wa2"], np.float32)),
        "ba2": np.ascontiguousarray(np.asarray(inputs["ba2"], np.float32)),
        "Wg": np.ascontiguousarray(np.asarray(inputs["Wg"], np.float32)),
        "bg": np.ascontiguousarray(np.asarray(inputs["bg"], np.float32)),
        "scale_col": np.full((128, 1), 1.0 / 8192.0, np.float32),
    }
    in_maps = []
    for d in range(NCORES):
        m = dict(common)
        m["ks"] = np.ascontiguousarray(k[d * SHARD : (d + 1) * SHARD])
        if use_collective:
            m["qs"] = np.ascontiguousarray(q[d * SHARD : (d + 1) * SHARD])
        else:
            m["qs"] = q
        in_maps.append(m)
    return in_maps


def _run(inputs, use_collective: bool, trace: bool = False):
    from concourse.bass_utils import run_bass_kernel_spmd

    nc = _get_prog(use_collective)
    in_maps = _make_in_maps(inputs, use_collective)
    res = run_bass_kernel_spmd(
        nc, in_maps, core_ids=list(range(NCORES)), trace=trace
    )
    full = np.empty((1, 1, SEQ, SEQ), np.float32)
    for d in range(NCORES):
        full[0, 0, :, d * SHARD : (d + 1) * SHARD] = res.results[d]["out"]
    return full, res


def kernel(**inputs) -> np.ndarray:
    use_collective = os.environ.get("CA1_NO_COLLECTIVE", "0") != "1"
    if use_collective:
        for attempt in range(2):
            try:
                full, _ = _run(inputs, True)
                return full
            except Exception:
                _PROG_CACHE.pop(True, None)
        # fall back to the zero-communication variant (replicated query)
    full, _ = _run(inputs, False)
    return full


# revision 26
# speedup vs baseline: 1.0332x; 1.0332x over previous
"""Trainium2 Bass kernel for nn_CA1AttentionGate.

Computes, for full inputs (B=1, S=8192, H=1024, F=128, K=2):
    temporal = relu(t @ Wt1 + bt1) @ Wt2 + bt2          [K,F]
    mem      = dg_features + temporal                    [K,F]
    qmean    = query.mean(axis=1)                        [1,H]
    score_k  = tanh([mem_k ; qmean] @ Wa1 + ba1) @ Wa2 + ba2
    w_k      = sigmoid(score_k)
    g_k      = mem_k @ Wg + bg                           [K,H]
    row[s]   = (1/K) * sum_k w_k * (g_k . key[s])        [S]
    out      = broadcast(row) -> [1,1,S,S]

Sharding: sequence-parallel over the key/seq axis across 8 cores.  Each
core computes the final gate row for its 1024 key positions and writes
its dense [8192, 1024] column slab of the output.  The slab is written
in fp16 (well within the 2e-2 tolerance; the host upcasts on gather),
halving the dominant output-write traffic.  The only cross-core
quantity is qmean: each core reduces its query shard via PE matmuls
into PSUM and a 4KB AllGather completes the mean (fallback variant
replicates the full query read instead).
"""

import os

import numpy as np

SEQ = 8192
H = 1024
F = 128
K = 2
NCORES = 8
SHARD = SEQ // NCORES  # 1024
NT = SHARD // 128  # 8 key tiles per shard

_PROG_CACHE = {}


def _build(use_collective: bool):
    import concourse.bacc as bacc
    import concourse.bass as bass
    import concourse.tile as tile
    from concourse import mybir
    from concourse.tile_rust import add_dep_helper

    AF = mybir.ActivationFunctionType
    ALU = mybir.AluOpType
    f32 = mybir.dt.float32
    f32r = mybir.dt.float32r
    f16 = mybir.dt.float16

    nc = bacc.Bacc(
        "TRN2",
        target_bir_lowering=False,
        debug=False,
        num_devices=NCORES,
    )

    def din(name, shape, dt=None):
        return nc.dram_tensor(
            name, list(shape), dt or f32, kind="ExternalInput"
        ).ap()

    q_rows = SHARD if use_collective else SEQ
    # f32r: same bits as f32, but 4x faster PE matmuls (plenty of precision
    # for the 2e-2 tolerance)
    qs = din("qs", (q_rows, H), f32r)
    ks = din("ks", (SHARD, H))
    dg = din("dg", (K, F))
    ts = din("ts", (K,))
    Wt1 = din("Wt1", (1, F // 4))
    bt1 = din("bt1", (F // 4,))
    Wt2 = din("Wt2", (F // 4, F))
    bt2 = din("bt2", (F,))
    Wa1 = din("Wa1", (F + H, F))
    ba1 = din("ba1", (F,))
    Wa2 = din("Wa2", (F, 1))
    ba2 = din("ba2", (1,))
    Wg = din("Wg", (F, H), f32r)
    bg = din("bg", (H,))
    # column of 1/SEQ: the qsum partition-reduce matmul yields the scaled
    # mean contribution directly
    scale_col = din("scale_col", (128, 1), f32r)
    out = nc.dram_tensor("out", [SEQ, SHARD], f16, kind="ExternalOutput").ap()

    def bcast(ap, n):
        # replicate a DRAM row across n partitions (stride-0 partition dim)
        return bass.AP(tensor=ap.tensor, offset=ap.offset, ap=[[0, n]] + list(ap.ap))

    def col(ap, n):
        # load a flat [n] DRAM vector as an [n, 1] column
        return bass.AP(tensor=ap.tensor, offset=ap.offset, ap=[[1, n], [n, 1]])

    with tile.TileContext(nc) as tc:
        with (
            tc.tile_pool(name="consts", bufs=1) as cp,
            tc.tile_pool(name="work", bufs=1) as wp,
            tc.tile_pool(name="qstream", bufs=8) as qp,
            tc.tile_pool(name="scratch", bufs=2) as sp,
            tc.tile_pool(name="psum_small", bufs=1, space="PSUM") as pps,
            tc.tile_pool(name="psum_big", bufs=2, space="PSUM") as ppb,
            tc.tile_pool(name="dram", bufs=1, space="DRAM") as dp,
        ):
            # ---- scale column first (feeds the qsum matmuls) ------------
            sc_c = cp.tile([128, 1], f32r, tag="sc")
            nc.sync.dma_start(sc_c, scale_col)
            # gate/temporal weights ride the (idle) ACT queue so the query
            # stream owns the sync queue from t=0; the g-vector chain
            # (memT -> g matmuls -> gb broadcasts) is ready before keys land
            Wg_sb = cp.tile([F, H], f32r, tag="Wg")
            nc.scalar.dma_start(Wg_sb, Wg)
            bg_sb = cp.tile([1, H], f32, tag="bg")
            nc.scalar.dma_start(bg_sb, bg.rearrange("(a h) -> a h", a=1))
            Wt2_sb = cp.tile([F // 4, F], f32, tag="Wt2")
            nc.scalar.dma_start(Wt2_sb, Wt2)
            dgT_sb = cp.tile([F, K], f32, tag="dgT")
            nc.scalar.dma_start(dgT_sb, dg.rearrange("k f -> f k"))
            tb_sb = cp.tile([F // 4, K], f32, tag="tb")
            nc.scalar.dma_start(tb_sb, bcast(ts, F // 4))
            Wt1T_sb = cp.tile([F // 4, 1], f32, tag="Wt1T")
            nc.scalar.dma_start(Wt1T_sb, col(Wt1, F // 4))
            bt1T_sb = cp.tile([F // 4, 1], f32, tag="bt1T")
            nc.scalar.dma_start(bt1T_sb, col(bt1, F // 4))
            bt2T_sb = cp.tile([F, 1], f32, tag="bt2T")
            nc.scalar.dma_start(bt2T_sb, col(bt2, F))
            # ---- query shard DMAs get the wire next --------------------
            nq = q_rows // 128
            qv = qs.rearrange("(t p) h -> t p h", p=128)
            qtiles = []
            q_insts = []
            for i in range(nq):
                qt = qp.tile([128, H], f32r, tag="qt")
                q_insts.append(nc.sync.dma_start(qt, qv[i]))
                qtiles.append(qt)


            # warm the ACT function tables used late in the critical path
            warm1 = cp.tile([1, 1], f32, tag="warm1")
            nc.scalar.activation(warm1, sc_c[0:1, :], AF.Tanh)
            warm2 = cp.tile([1, 1], f32, tag="warm2")
            nc.scalar.activation(warm2, sc_c[0:1, :], AF.Sigmoid)

            # key shard: interleaved, ktiles[j][p, :] = ks[p*NT + j, :];
            # explicitly ordered after the small-weight block
            kv = ks.rearrange("(p t) h -> p t h", t=NT)
            ktiles = []
            k_insts = []
            for j in range(NT):
                kt = cp.tile([128, H], f32, tag=f"ks{j}")
                ki = nc.sync.dma_start(kt, kv[:, j, :])
                add_dep_helper(ki.ins, q_insts[-1].ins,
                               reason="key reads after query stream")
                ktiles.append(kt)
                k_insts.append(ki)

            # ---- scorer weights (needed only post-collective) ----------
            Wa1m_sb = cp.tile([128, 128], f32, tag="Wa1m")
            wb0 = nc.sync.dma_start(Wa1m_sb, Wa1[0:F, :])
            add_dep_helper(wb0.ins, k_insts[-1].ins,
                           reason="scorer weights after key stream")
            # qmean rows of Wa1 re-paired to the interleaved qmT layout:
            # chunk c pairs with rows {128 + i*8 + c}
            Wa1q_sb = cp.tile([128, 8, 128], f32, tag="Wa1q")
            nc.sync.dma_start(
                Wa1q_sb, Wa1[F : F + H, :].rearrange("(i c) f -> i c f", c=8)
            )
            Wa2_sb = cp.tile([F, 1], f32, tag="Wa2")
            nc.sync.dma_start(Wa2_sb, Wa2)
            ba1T_sb = cp.tile([F, 1], f32, tag="ba1T")
            nc.sync.dma_start(ba1T_sb, col(ba1, F))
            ba2b_sb = cp.tile([1, 1], f32, tag="ba2b")
            nc.sync.dma_start(ba2b_sb, bcast(ba2, 1))

            # ---- qsum on PE: psum[0, h] = sum_s q[s, h] / SEQ -----------
            # (accumulating matmuls keep DVE free for the matvec)
            # psum[0, h] = sum_s q[s, h] / SEQ — accumulating f32r matmuls,
            # one pair per query tile as it lands (keeps DVE free)
            qsum_ps = ppb.tile([1, H], f32, tag="big")
            for i in range(nq):
                nc.tensor.matmul(
                    qsum_ps[:, 0:512], lhsT=sc_c, rhs=qtiles[i][:, 0:512],
                    start=(i == 0), stop=(i == nq - 1),
                )
                nc.tensor.matmul(
                    qsum_ps[:, 512:1024], lhsT=sc_c, rhs=qtiles[i][:, 512:1024],
                    start=(i == 0), stop=(i == nq - 1),
                )
            qpart_sb = wp.tile([1, H], f32, tag="qpart")
            nc.scalar.copy(qpart_sb, qsum_ps)
            if use_collective:
                cc_in = dp.tile([1, H], f32, tag="ccin")
                cc_out = dp.tile([NCORES, H], f32, tag="ccout")
                nc.scalar.dma_start(cc_in, qpart_sb)
                nc.gpsimd.collective_compute(
                    "AllGather",
                    ALU.bypass,
                    replica_groups=[list(range(NCORES))],
                    ins=[cc_in.opt()],
                    outs=[cc_out.opt()],
                )

            # ---- temporal MLP -> memT [F, K] ---------------------------
            h1T = wp.tile([F // 4, K], f32, tag="h1T")
            nc.vector.tensor_scalar_mul(h1T, tb_sb, Wt1T_sb)
            nc.vector.tensor_scalar_add(h1T, h1T, bt1T_sb)
            nc.vector.tensor_relu(h1T, h1T)
            tT_ps = pps.tile([F, K], f32, tag="small")
            nc.tensor.matmul(tT_ps, lhsT=Wt2_sb, rhs=h1T, start=True, stop=True)
            memT_sb = wp.tile([F, K], f32, tag="memT")
            nc.scalar.activation(memT_sb, tT_ps, AF.Identity, bias=bt2T_sb, scale=1.0)
            nc.vector.tensor_add(memT_sb, memT_sb, dgT_sb)
            memTr_sb = wp.tile([F, K], f32r, tag="memTr")
            nc.vector.tensor_copy(memTr_sb, memT_sb)

            # ---- gate rows g_k = mem_k @ Wg + bg  [1, H] ---------------
            def g_row(k):
                g_ps = ppb.tile([1, H], f32, tag="big")
                nc.tensor.matmul(g_ps[:, 0:512], lhsT=memTr_sb[:, k : k + 1],
                                 rhs=Wg_sb[:, 0:512], start=True, stop=True)
                nc.tensor.matmul(g_ps[:, 512:1024], lhsT=memTr_sb[:, k : k + 1],
                                 rhs=Wg_sb[:, 512:1024], start=True, stop=True)
                return g_ps

            g0_ps = g_row(0)
            g0_sb = wp.tile([1, H], f32, tag="g0r")
            nc.vector.tensor_add(g0_sb, g0_ps, bg_sb)
            gb0 = wp.tile([128, H], f32, tag="gb0")
            nc.gpsimd.partition_broadcast(gb0[:, :], g0_sb[:, :])
            g1_ps = g_row(1)
            g1_sb = wp.tile([1, H], f32, tag="g1r")
            nc.vector.tensor_add(g1_sb, g1_ps, bg_sb)
            gb1 = wp.tile([128, H], f32, tag="gb1")
            nc.gpsimd.partition_broadcast(gb1[:, :], g1_sb[:, :])

            # ---- matvec: fused mul+reduce on DVE -----------------------
            # rcc[p, j, k] = sum_h g_k[h] * ks[p*NT+j, h]
            # (NB: tensor_tensor_reduce would fuse mul+reduce on DVE but
            # crashes real TRN2 hardware — keep mul + separate reduce.)
            # Split the 16 muls and 16 reductions across DVE/Pool/ACT so no
            # single engine's serial stream exceeds the key-load window.
            rcc = wp.tile([128, NT, K], f32, tag="rcc")
            pool_mul = {(1, 1), (3, 1), (4, 1), (5, 1), (7, 1)}
            dve_red = {(3, 1), (4, 1), (5, 1), (7, 0), (7, 1)}
            last_dve = last_act = None
            for j in range(NT):
                for k, gb in ((0, gb0), (1, gb1)):
                    if (j, k) in pool_mul:
                        prod = sp.tile([128, H], f32, tag="prodp")
                        nc.gpsimd.tensor_mul(prod, ktiles[j], gb)
                    else:
                        prod = sp.tile([128, H], f32, tag="prodv")
                        last_dve = nc.vector.tensor_mul(prod, ktiles[j], gb)
                    if (j, k) in dve_red:
                        last_dve = nc.vector.tensor_reduce(
                            rcc[:, j, k : k + 1], prod,
                            axis=mybir.AxisListType.X, op=ALU.add,
                        )
                    else:
                        junk = sp.tile([128, H], f32, tag="junk")
                        last_act = nc.scalar.activation(
                            junk, prod, AF.Copy,
                            accum_out=rcc[:, j, k : k + 1],
                        )

            if use_collective:
                # gather-result load parked late on the sync ring (must not
                # block the ACT accum stream behind the collective wait)
                qmTd8 = wp.tile([128, NCORES, 8], f32, tag="qmTd8")
                nc.sync.dma_start(
                    qmTd8, cc_out[:, :].rearrange("d (p c) -> p d c", c=8)
                )


            # ---- post-collective: qmT, scorer, weights -----------------
            # (on Pool — a DVE/ACT placement would park a collective-gated
            # wait in the middle of the in-order matvec streams)
            # qmT[p, c] = qmean[p*8 + c]  (interleaved reshape layout)
            qmT = wp.tile([128, 8], f32, tag="qmT")
            if use_collective:
                # sum gathered partials over d ([p, c, d] view, reduce X);
                # pinned after the matvec so its collective-gated wait cannot
                # stall the in-order DVE stream mid-matvec
                qr = nc.vector.tensor_reduce(
                    qmT, qmTd8[:, :, :].rearrange("p d c -> p c d"),
                    axis=mybir.AxisListType.X, op=ALU.add,
                )
                add_dep_helper(qr.ins, last_dve.ins,
                               reason="qmT reduce after matvec on DVE")
            else:
                nc.scalar.dma_start(qmT, qpart_sb[:, :])
            qmTd = wp.tile([128, 8, K], f32, tag="qmTd")
            nc.gpsimd.tensor_copy(qmTd[:, :, 0:1], qmT[:, :].rearrange("p c -> p c ()"))
            nc.gpsimd.tensor_copy(qmTd[:, :, 1:2], qmT[:, :].rearrange("p c -> p c ()"))
            haT_ps = pps.tile([F, K], f32, tag="haT")
            nc.tensor.matmul(haT_ps, lhsT=Wa1m_sb, rhs=memT_sb,
                             start=True, stop=False)
            for c in range(8):
                nc.tensor.matmul(haT_ps, lhsT=Wa1q_sb[:, c, :],
                                 rhs=qmTd[:, c, :], start=False, stop=(c == 7))
            aT_sb = wp.tile([F, K], f32, tag="aT")
            th = nc.scalar.activation(aT_sb, haT_ps, AF.Tanh, bias=ba1T_sb, scale=1.0)
            add_dep_helper(th.ins, last_act.ins,
                           reason="tanh after matvec accums on ACT")
            scoreT_ps = pps.tile([1, K], f32, tag="small")
            nc.tensor.matmul(scoreT_ps, lhsT=Wa2_sb, rhs=aT_sb, start=True, stop=True)
            wvT_sb = wp.tile([1, K], f32, tag="wvT")
            nc.scalar.activation(wvT_sb, scoreT_ps, AF.Sigmoid, bias=ba2b_sb, scale=1.0)
            nc.scalar.mul(wvT_sb, wvT_sb, 1.0 / K)
            wvb = wp.tile([128, K], f32, tag="wvb")
            nc.gpsimd.partition_broadcast(wvb[:, :], wvT_sb[:, :])

            # ---- combine anchors in the tiny [128, NT] layout ----------
            o_t8 = wp.tile([128, NT], f32, tag="o_t8")
            nc.vector.tensor_scalar_mul(o_t8, rcc[:, :, 1], wvb[:, 1:2])
            o128 = wp.tile([128, NT], f16, tag="o128")
            nc.vector.scalar_tensor_tensor(
                o128, rcc[:, :, 0], wvb[:, 0:1], o_t8, ALU.mult, ALU.add
            )
            # row[0, p*NT + j] = o128[p, j]  (partition-major flatten = s)
            o_row = wp.tile([1, SHARD], f16, tag="o_row")
            nc.scalar.dma_start(o_row[:, :], o128[:, :])
            out_sb = wp.tile([128, SHARD], f16, tag="osb")
            nc.gpsimd.partition_broadcast(out_sb[:, :], o_row[:, :])

            # ---- output: 64 x [128 rows, SHARD cols], all rows = row ---
            outv = out.rearrange("(b p) n -> b p n", p=128)
            for b in range(SEQ // 128):
                nc.sync.dma_start(outv[b], out_sb)

    nc.compile()
    return nc


def _get_prog(use_collective: bool):
    key = bool(use_collective)
    if key not in _PROG_CACHE:
        _PROG_CACHE[key] = _build(key)
    return _PROG_CACHE[key]


def _make_in_maps(inputs, use_collective: bool):
    q = np.ascontiguousarray(np.asarray(inputs["query"], np.float32)[0])  # [S,H]
    k = np.ascontiguousarray(np.asarray(inputs["key"], np.float32)[0])  # [S,H]
    common = {
        "dg": np.ascontiguousarray(np.asarray(inputs["dg_features"], np.float32)),
        "ts": np.ascontiguousarray(np.asarray(inputs["timestamps"], np.float32)),
        "Wt1": np.ascontiguousarray(np.asarray(inputs["Wt1"], np.float32)),
        "bt1": np.ascontiguousarray(np.asarray(inputs["bt1"], np.float32)),
        "Wt2": np.ascontiguousarray(np.asarray(inputs["Wt2"], np.float32)),
        "bt2": np.ascontiguousarray(np.asarray(inputs["bt2"], np.float32)),
        "Wa1": np.ascontiguousarray(np.asarray(inputs["Wa1"], np.float32)),
        "ba1": np.ascontiguousarray(np.asarray(inputs["ba1"], np.float32)),
        "Wa2": np.ascontiguousarray(np.asarray(inputs["Wa2"], np.float32)),
        "ba2": np.ascontiguousarray(np.asarray(inputs["ba2"], np.float32)),
        "Wg": np.ascontiguousarray(np.asarray(inputs["Wg"], np.float32)),
        "bg": np.ascontiguousarray(np.asarray(inputs["bg"], np.float32)),
        "scale_col": np.full((128, 1), 1.0 / 8192.0, np.float32),
    }
    in_maps = []
    for d in range(NCORES):
        m = dict(common)
        m["ks"] = np.ascontiguousarray(k[d * SHARD : (d + 1) * SHARD])
        if use_collective:
            m["qs"] = np.ascontiguousarray(q[d * SHARD : (d + 1) * SHARD])
        else:
            m["qs"] = q
        in_maps.append(m)
    return in_maps


def _run(inputs, use_collective: bool, trace: bool = False):
    from concourse.bass_utils import run_bass_kernel_spmd

    nc = _get_prog(use_collective)
    in_maps = _make_in_maps(inputs, use_collective)
    res = run_bass_kernel_spmd(
        nc, in_maps, core_ids=list(range(NCORES)), trace=trace
    )
    full = np.empty((1, 1, SEQ, SEQ), np.float32)
    for d in range(NCORES):
        full[0, 0, :, d * SHARD : (d + 1) * SHARD] = res.results[d]["out"]
    return full, res


def kernel(**inputs) -> np.ndarray:
    use_collective = os.environ.get("CA1_NO_COLLECTIVE", "0") != "1"
    if use_collective:
        for attempt in range(2):
            try:
                full, _ = _run(inputs, True)
                return full
            except Exception:
                _PROG_CACHE.pop(True, None)
        # fall back to the zero-communication variant (replicated query)
    full, _ = _run(inputs, False)
    return full


# revision 33
# speedup vs baseline: 1.0789x; 1.0442x over previous
"""Trainium2 Bass kernel for nn_CA1AttentionGate.

Computes, for full inputs (B=1, S=8192, H=1024, F=128, K=2):
    temporal = relu(t @ Wt1 + bt1) @ Wt2 + bt2          [K,F]
    mem      = dg_features + temporal                    [K,F]
    qmean    = query.mean(axis=1)                        [1,H]
    score_k  = tanh([mem_k ; qmean] @ Wa1 + ba1) @ Wa2 + ba2
    w_k      = sigmoid(score_k)
    g_k      = mem_k @ Wg + bg                           [K,H]
    row[s]   = (1/K) * sum_k w_k * (g_k . key[s])        [S]
    out      = broadcast(row) -> [1,1,S,S]

Sharding: sequence-parallel over the key/seq axis across 8 cores.  Each
core computes the final gate row for its 1024 key positions and writes
its dense [8192, 1024] column slab of the output.  The slab is written
in fp16 (well within the 2e-2 tolerance; the host upcasts on gather),
halving the dominant output-write traffic.  The only cross-core
quantity is qmean: each core reduces its query shard via PE matmuls
into PSUM and a 4KB AllGather completes the mean (fallback variant
replicates the full query read instead).
"""

import os

import numpy as np

SEQ = 8192
H = 1024
F = 128
K = 2
NCORES = 8
SHARD = SEQ // NCORES  # 1024
NT = SHARD // 128  # 8 key tiles per shard

_PROG_CACHE = {}


def _build(use_collective: bool):
    import concourse.bacc as bacc
    import concourse.bass as bass
    import concourse.tile as tile
    from concourse import mybir
    from concourse.tile_rust import add_dep_helper

    AF = mybir.ActivationFunctionType
    ALU = mybir.AluOpType
    f32 = mybir.dt.float32
    f32r = mybir.dt.float32r
    f16 = mybir.dt.float16

    nc = bacc.Bacc(
        "TRN2",
        target_bir_lowering=False,
        debug=False,
        num_devices=NCORES,
    )

    def din(name, shape, dt=None):
        return nc.dram_tensor(
            name, list(shape), dt or f32, kind="ExternalInput"
        ).ap()

    q_rows = SHARD if use_collective else SEQ
    # f32r: same bits as f32, but 4x faster PE matmuls (plenty of precision
    # for the 2e-2 tolerance)
    qs = din("qs", (q_rows, H), f32r)
    ks = din("ks", (SHARD, H))
    dg = din("dg", (K, F))
    ts = din("ts", (K,))
    Wt1 = din("Wt1", (1, F // 4))
    bt1 = din("bt1", (F // 4,))
    Wt2 = din("Wt2", (F // 4, F))
    bt2 = din("bt2", (F,))
    Wa1 = din("Wa1", (F + H, F))
    ba1 = din("ba1", (F,))
    Wa2 = din("Wa2", (F, 1))
    ba2 = din("ba2", (1,))
    Wg = din("Wg", (F, H), f32r)
    bg = din("bg", (H,))
    # column of 1/SEQ: the qsum partition-reduce matmul yields the scaled
    # mean contribution directly
    scale_col = din("scale_col", (128, 1), f32r)
    out = nc.dram_tensor("out", [SEQ, SHARD], f16, kind="ExternalOutput").ap()

    def bcast(ap, n):
        # replicate a DRAM row across n partitions (stride-0 partition dim)
        return bass.AP(tensor=ap.tensor, offset=ap.offset, ap=[[0, n]] + list(ap.ap))

    def col(ap, n):
        # load a flat [n] DRAM vector as an [n, 1] column
        return bass.AP(tensor=ap.tensor, offset=ap.offset, ap=[[1, n], [n, 1]])

    with tile.TileContext(nc) as tc:
        with (
            tc.tile_pool(name="consts", bufs=1) as cp,
            tc.tile_pool(name="work", bufs=1) as wp,
            tc.tile_pool(name="qstream", bufs=8) as qp,
            tc.tile_pool(name="scratch", bufs=4) as sp,
            tc.tile_pool(name="psum_small", bufs=1, space="PSUM") as pps,
            tc.tile_pool(name="psum_big", bufs=2, space="PSUM") as ppb,
            tc.tile_pool(name="dram", bufs=1, space="DRAM") as dp,
        ):
            # ---- scale column first (feeds the qsum matmuls) ------------
            sc_c = cp.tile([128, 1], f32r, tag="sc")
            nc.sync.dma_start(sc_c, scale_col)
            # gate/temporal weights ride the (idle) ACT queue so the query
            # stream owns the sync queue from t=0; the g-vector chain
            # (memT -> g matmuls -> gb broadcasts) is ready before keys land
            Wt2_sb = cp.tile([F // 4, F], f32, tag="Wt2")
            nc.scalar.dma_start(Wt2_sb, Wt2)
            dgT_sb = cp.tile([F, K], f32, tag="dgT")
            nc.scalar.dma_start(dgT_sb, dg.rearrange("k f -> f k"))
            tb_sb = cp.tile([F // 4, K], f32, tag="tb")
            nc.scalar.dma_start(tb_sb, bcast(ts, F // 4))
            Wt1T_sb = cp.tile([F // 4, 1], f32, tag="Wt1T")
            nc.scalar.dma_start(Wt1T_sb, col(Wt1, F // 4))
            bt1T_sb = cp.tile([F // 4, 1], f32, tag="bt1T")
            nc.scalar.dma_start(bt1T_sb, col(bt1, F // 4))
            bt2T_sb = cp.tile([F, 1], f32, tag="bt2T")
            nc.scalar.dma_start(bt2T_sb, col(bt2, F))
            # warm the ACT function tables used late in the critical path
            warm1 = cp.tile([1, 1], f32, tag="warm1")
            nc.scalar.activation(warm1, sc_c[0:1, :], AF.Tanh)
            warm2 = cp.tile([1, 1], f32, tag="warm2")
            nc.scalar.activation(warm2, sc_c[0:1, :], AF.Sigmoid)

            # stream order on the (in-order) sync queue:
            #  collective: query -> Wg/bg -> keys (collective path is the
            #    critical chain, so the query shard owns the wire first)
            #  fallback:   keys -> Wg/bg -> query (the 93us replicated query
            #    read dominates; the whole matvec hides under it)
            nq = q_rows // 128
            qv = qs.rearrange("(t p) h -> t p h", p=128)
            kv = ks.rearrange("(p t) h -> p t h", t=NT)

            def load_query():
                tiles, insts = [], []
                for i in range(nq):
                    qt = qp.tile([128, H], f32r, tag="qt")
                    insts.append(nc.sync.dma_start(qt, qv[i]))
                    tiles.append(qt)
                return tiles, insts

            def load_wg():
                Wg_sb = cp.tile([F, H], f32r, tag="Wg")
                nc.sync.dma_start(Wg_sb, Wg)
                bg_sb = cp.tile([1, H], f32, tag="bg")
                nc.sync.dma_start(bg_sb, bg.rearrange("(a h) -> a h", a=1))
                return Wg_sb, bg_sb

            def load_key(j):
                # interleaved: ktiles[j][p, :] = ks[p*NT + j, :]
                kt = cp.tile([128, H], f32, tag=f"ks{j}")
                return kt, nc.sync.dma_start(kt, kv[:, j, :])

            def qsum_block(qtiles):
                # psum[0, h] = sum_s q[s, h] / SEQ — accumulating f32r
                # matmuls, one pair per query tile (keeps DVE free)
                qsum_ps = ppb.tile([1, H], f32, tag="big")
                for i in range(len(qtiles)):
                    nc.tensor.matmul(
                        qsum_ps[:, 0:512], lhsT=sc_c, rhs=qtiles[i][:, 0:512],
                        start=(i == 0), stop=(i == len(qtiles) - 1),
                    )
                    nc.tensor.matmul(
                        qsum_ps[:, 512:1024], lhsT=sc_c,
                        rhs=qtiles[i][:, 512:1024],
                        start=(i == 0), stop=(i == len(qtiles) - 1),
                    )
                qpart_sb = wp.tile([1, H], f32, tag="qpart")
                nc.scalar.copy(qpart_sb, qsum_ps)
                return qpart_sb

            if use_collective:
                qtiles, q_insts = load_query()
                Wg_sb, bg_sb = load_wg()
                # key tile 0, then a reserved wire slot for the 4KB cc_in
                # store (on the in-order sync queue it would otherwise be
                # served only after the whole key stream)
                ktiles, k_insts = [], []
                kt, ki = load_key(0)
                ktiles.append(kt)
                k_insts.append(ki)
                qpart_sb = qsum_block(qtiles)
                cc_in = dp.tile([1, H], f32, tag="ccin")
                nc.sync.dma_start(cc_in, qpart_sb)
                for j in range(1, NT):
                    kt, ki = load_key(j)
                    ktiles.append(kt)
                    k_insts.append(ki)
                last_stream = k_insts[-1]
            else:
                ktiles, k_insts = [], []
                for j in range(NT):
                    kt, ki = load_key(j)
                    ktiles.append(kt)
                    k_insts.append(ki)
                Wg_sb, bg_sb = load_wg()
                qtiles, q_insts = load_query()
                qpart_sb = qsum_block(qtiles)
                last_stream = q_insts[-1]

            # ---- scorer weights (needed only post-collective) ----------
            Wa1m_sb = cp.tile([128, 128], f32, tag="Wa1m")
            wb0 = nc.sync.dma_start(Wa1m_sb, Wa1[0:F, :])
            add_dep_helper(wb0.ins, last_stream.ins,
                           reason="scorer weights after the big streams")
            # qmean rows of Wa1 re-paired to the interleaved qmT layout:
            # chunk c pairs with rows {128 + i*8 + c}
            Wa1q_sb = cp.tile([128, 8, 128], f32, tag="Wa1q")
            nc.sync.dma_start(
                Wa1q_sb, Wa1[F : F + H, :].rearrange("(i c) f -> i c f", c=8)
            )
            Wa2_sb = cp.tile([F, 1], f32, tag="Wa2")
            nc.sync.dma_start(Wa2_sb, Wa2)
            ba1T_sb = cp.tile([F, 1], f32, tag="ba1T")
            nc.sync.dma_start(ba1T_sb, col(ba1, F))
            ba2b_sb = cp.tile([1, 1], f32, tag="ba2b")
            nc.sync.dma_start(ba2b_sb, bcast(ba2, 1))

            if use_collective:
                cc_out = dp.tile([NCORES, H], f32, tag="ccout")
                nc.gpsimd.collective_compute(
                    "AllGather",
                    ALU.bypass,
                    replica_groups=[list(range(NCORES))],
                    ins=[cc_in.opt()],
                    outs=[cc_out.opt()],
                )

            # ---- temporal MLP -> memT [F, K] ---------------------------
            h1T = wp.tile([F // 4, K], f32, tag="h1T")
            nc.vector.tensor_scalar_mul(h1T, tb_sb, Wt1T_sb)
            nc.vector.tensor_scalar_add(h1T, h1T, bt1T_sb)
            nc.vector.tensor_relu(h1T, h1T)
            tT_ps = pps.tile([F, K], f32, tag="small")
            nc.tensor.matmul(tT_ps, lhsT=Wt2_sb, rhs=h1T, start=True, stop=True)
            memT_sb = wp.tile([F, K], f32, tag="memT")
            nc.scalar.activation(memT_sb, tT_ps, AF.Identity, bias=bt2T_sb, scale=1.0)
            nc.vector.tensor_add(memT_sb, memT_sb, dgT_sb)
            memTr_sb = wp.tile([F, K], f32r, tag="memTr")
            nc.vector.tensor_copy(memTr_sb, memT_sb)

            # ---- gate rows g_k = mem_k @ Wg + bg  [1, H] ---------------
            def g_row(k):
                g_ps = ppb.tile([1, H], f32, tag="big")
                nc.tensor.matmul(g_ps[:, 0:512], lhsT=memTr_sb[:, k : k + 1],
                                 rhs=Wg_sb[:, 0:512], start=True, stop=True)
                nc.tensor.matmul(g_ps[:, 512:1024], lhsT=memTr_sb[:, k : k + 1],
                                 rhs=Wg_sb[:, 512:1024], start=True, stop=True)
                return g_ps

            g0_ps = g_row(0)
            g0_sb = wp.tile([1, H], f32, tag="g0r")
            nc.vector.tensor_add(g0_sb, g0_ps, bg_sb)
            gb0 = wp.tile([128, H], f32, tag="gb0")
            nc.gpsimd.partition_broadcast(gb0[:, :], g0_sb[:, :])
            g1_ps = g_row(1)
            g1_sb = wp.tile([1, H], f32, tag="g1r")
            nc.vector.tensor_add(g1_sb, g1_ps, bg_sb)
            gb1 = wp.tile([128, H], f32, tag="gb1")
            nc.gpsimd.partition_broadcast(gb1[:, :], g1_sb[:, :])

            # ---- matvec: fused mul+reduce on DVE -----------------------
            # rcc[p, j, k] = sum_h g_k[h] * ks[p*NT+j, h]
            # (NB: tensor_tensor_reduce would fuse mul+reduce on DVE but
            # crashes real TRN2 hardware — keep mul + separate reduce.)
            # Split the 16 muls and 16 reductions across DVE/Pool/ACT so no
            # single engine's serial stream exceeds the key-load window.
            rcc = wp.tile([128, NT, K], f32, tag="rcc")
            pool_mul = {(1, 1), (3, 1), (4, 1), (5, 1), (7, 1)}
            dve_red = {(3, 1), (4, 1), (5, 1), (7, 0), (7, 1)}
            last_dve = last_act = None
            for j in range(NT):
                for k, gb in ((0, gb0), (1, gb1)):
                    if (j, k) in pool_mul:
                        prod = sp.tile([128, H], f32, tag="prodp")
                        nc.gpsimd.tensor_mul(prod, ktiles[j], gb)
                    else:
                        prod = sp.tile([128, H], f32, tag="prodv")
                        last_dve = nc.vector.tensor_mul(prod, ktiles[j], gb)
                    if (j, k) in dve_red:
                        last_dve = nc.vector.tensor_reduce(
                            rcc[:, j, k : k + 1], prod,
                            axis=mybir.AxisListType.X, op=ALU.add,
                        )
                    else:
                        junk = sp.tile([128, H], f32, tag="junk")
                        last_act = nc.scalar.activation(
                            junk, prod, AF.Copy,
                            accum_out=rcc[:, j, k : k + 1],
                        )

            if use_collective:
                # gather-result load parked late on the sync ring (must not
                # block the ACT accum stream behind the collective wait)
                qmTd8 = wp.tile([128, NCORES, 8], f32, tag="qmTd8")
                nc.sync.dma_start(
                    qmTd8, cc_out[:, :].rearrange("d (p c) -> p d c", c=8)
                )


            # ---- post-collective: qmT, scorer, weights -----------------
            # (on Pool — a DVE/ACT placement would park a collective-gated
            # wait in the middle of the in-order matvec streams)
            # qmT[p, c] = qmean[p*8 + c]  (interleaved reshape layout)
            qmT = wp.tile([128, 8], f32, tag="qmT")
            if use_collective:
                # sum gathered partials over d ([p, c, d] view, reduce X);
                # pinned after the matvec so its collective-gated wait cannot
                # stall the in-order DVE stream mid-matvec
                qr = nc.vector.tensor_reduce(
                    qmT, qmTd8[:, :, :].rearrange("p d c -> p c d"),
                    axis=mybir.AxisListType.X, op=ALU.add,
                )
                add_dep_helper(qr.ins, last_dve.ins,
                               reason="qmT reduce after matvec on DVE")
            else:
                nc.scalar.dma_start(qmT, qpart_sb[:, :])
            # mem part of the scorer pre-activation (anchor-dependent)
            haT_ps = pps.tile([F, K], f32, tag="haT")
            nc.tensor.matmul(haT_ps, lhsT=Wa1m_sb, rhs=memT_sb,
                             start=True, stop=True)
            # qmean part is identical for both anchors -> [F, 1], applied as
            # the tanh bias together with ba1 (saves the anchor-duplication)
            hq_ps = pps.tile([F, 1], f32, tag="hq")
            for c in range(8):
                nc.tensor.matmul(hq_ps, lhsT=Wa1q_sb[:, c, :],
                                 rhs=qmT[:, c : c + 1],
                                 start=(c == 0), stop=(c == 7))
            hqb = wp.tile([F, 1], f32, tag="hqb")
            nc.scalar.activation(hqb, hq_ps, AF.Identity, bias=ba1T_sb, scale=1.0)
            aT_sb = wp.tile([F, K], f32, tag="aT")
            th = nc.scalar.activation(aT_sb, haT_ps, AF.Tanh, bias=hqb, scale=1.0)
            add_dep_helper(th.ins, last_act.ins,
                           reason="tanh after matvec accums on ACT")
            scoreT_ps = pps.tile([1, K], f32, tag="small")
            nc.tensor.matmul(scoreT_ps, lhsT=Wa2_sb, rhs=aT_sb, start=True, stop=True)
            wvT_sb = wp.tile([1, K], f32, tag="wvT")
            # (1/K of the anchor mean is folded into Wg/bg host-side)
            nc.scalar.activation(wvT_sb, scoreT_ps, AF.Sigmoid, bias=ba2b_sb, scale=1.0)
            wvb = wp.tile([128, K], f32, tag="wvb")
            nc.gpsimd.partition_broadcast(wvb[:, :], wvT_sb[:, :])

            # ---- combine anchors in the tiny [128, NT] layout ----------
            o_t8 = wp.tile([128, NT], f32, tag="o_t8")
            nc.vector.tensor_scalar_mul(o_t8, rcc[:, :, 1], wvb[:, 1:2])
            o128 = wp.tile([128, NT], f16, tag="o128")
            nc.vector.scalar_tensor_tensor(
                o128, rcc[:, :, 0], wvb[:, 0:1], o_t8, ALU.mult, ALU.add
            )
            # row[0, p*NT + j] = o128[p, j]  (partition-major flatten = s)
            o_row = wp.tile([1, SHARD], f16, tag="o_row")
            nc.scalar.dma_start(o_row[:, :], o128[:, :])
            out_sb = wp.tile([128, SHARD], f16, tag="osb")
            nc.gpsimd.partition_broadcast(out_sb[:, :], o_row[:, :])

            # ---- output: 64 x [128 rows, SHARD cols], all rows = row ---
            outv = out.rearrange("(b p) n -> b p n", p=128)
            for b in range(SEQ // 128):
                nc.sync.dma_start(outv[b], out_sb)

    nc.compile()
    return nc


def _get_prog(use_collective: bool):
    key = bool(use_collective)
    if key not in _PROG_CACHE:
        _PROG_CACHE[key] = _build(key)
    return _PROG_CACHE[key]


def _make_in_maps(inputs, use_collective: bool):
    q = np.ascontiguousarray(np.asarray(inputs["query"], np.float32)[0])  # [S,H]
    k = np.ascontiguousarray(np.asarray(inputs["key"], np.float32)[0])  # [S,H]
    common = {
        "dg": np.ascontiguousarray(np.asarray(inputs["dg_features"], np.float32)),
        "ts": np.ascontiguousarray(np.asarray(inputs["timestamps"], np.float32)),
        "Wt1": np.ascontiguousarray(np.asarray(inputs["Wt1"], np.float32)),
        "bt1": np.ascontiguousarray(np.asarray(inputs["bt1"], np.float32)),
        "Wt2": np.ascontiguousarray(np.asarray(inputs["Wt2"], np.float32)),
        "bt2": np.ascontiguousarray(np.asarray(inputs["bt2"], np.float32)),
        "Wa1": np.ascontiguousarray(np.asarray(inputs["Wa1"], np.float32)),
        "ba1": np.ascontiguousarray(np.asarray(inputs["ba1"], np.float32)),
        "Wa2": np.ascontiguousarray(np.asarray(inputs["Wa2"], np.float32)),
        "ba2": np.ascontiguousarray(np.asarray(inputs["ba2"], np.float32)),
        "Wg": np.ascontiguousarray(np.asarray(inputs["Wg"], np.float32) / K),
        "bg": np.ascontiguousarray(np.asarray(inputs["bg"], np.float32) / K),
        "scale_col": np.full((128, 1), 1.0 / 8192.0, np.float32),
    }
    in_maps = []
    for d in range(NCORES):
        m = dict(common)
        m["ks"] = np.ascontiguousarray(k[d * SHARD : (d + 1) * SHARD])
        if use_collective:
            m["qs"] = np.ascontiguousarray(q[d * SHARD : (d + 1) * SHARD])
        else:
            m["qs"] = q
        in_maps.append(m)
    return in_maps


def _run(inputs, use_collective: bool, trace: bool = False):
    from concourse.bass_utils import run_bass_kernel_spmd

    nc = _get_prog(use_collective)
    in_maps = _make_in_maps(inputs, use_collective)
    res = run_bass_kernel_spmd(
        nc, in_maps, core_ids=list(range(NCORES)), trace=trace
    )
    full = np.empty((1, 1, SEQ, SEQ), np.float32)
    for d in range(NCORES):
        full[0, 0, :, d * SHARD : (d + 1) * SHARD] = res.results[d]["out"]
    return full, res


def kernel(**inputs) -> np.ndarray:
    use_collective = os.environ.get("CA1_NO_COLLECTIVE", "0") != "1"
    if use_collective:
        for attempt in range(2):
            try:
                full, _ = _run(inputs, True)
                return full
            except Exception:
                _PROG_CACHE.pop(True, None)
        # fall back to the zero-communication variant (replicated query)
    full, _ = _run(inputs, False)
    return full


# revision 34
# speedup vs baseline: 1.0803x; 1.0012x over previous
"""Trainium2 Bass kernel for nn_CA1AttentionGate.

Computes, for full inputs (B=1, S=8192, H=1024, F=128, K=2):
    temporal = relu(t @ Wt1 + bt1) @ Wt2 + bt2          [K,F]
    mem      = dg_features + temporal                    [K,F]
    qmean    = query.mean(axis=1)                        [1,H]
    score_k  = tanh([mem_k ; qmean] @ Wa1 + ba1) @ Wa2 + ba2
    w_k      = sigmoid(score_k)
    g_k      = mem_k @ Wg + bg                           [K,H]
    row[s]   = (1/K) * sum_k w_k * (g_k . key[s])        [S]
    out      = broadcast(row) -> [1,1,S,S]

Sharding: sequence-parallel over the key/seq axis across 8 cores.  Each
core computes the final gate row for its 1024 key positions and writes
its dense [8192, 1024] column slab of the output.  The slab is written
in fp16 (well within the 2e-2 tolerance; the host upcasts on gather),
halving the dominant output-write traffic.  The only cross-core
quantity is qmean: each core reduces its query shard via PE matmuls
into PSUM and a 4KB AllGather completes the mean (fallback variant
replicates the full query read instead).
"""

import os

import numpy as np

SEQ = 8192
H = 1024
F = 128
K = 2
NCORES = 8
SHARD = SEQ // NCORES  # 1024
NT = SHARD // 128  # 8 key tiles per shard

_PROG_CACHE = {}


def _build(use_collective: bool):
    import concourse.bacc as bacc
    import concourse.bass as bass
    import concourse.tile as tile
    from concourse import mybir
    from concourse.tile_rust import add_dep_helper

    AF = mybir.ActivationFunctionType
    ALU = mybir.AluOpType
    f32 = mybir.dt.float32
    f32r = mybir.dt.float32r
    f16 = mybir.dt.float16

    nc = bacc.Bacc(
        "TRN2",
        target_bir_lowering=False,
        debug=False,
        num_devices=NCORES,
    )

    def din(name, shape, dt=None):
        return nc.dram_tensor(
            name, list(shape), dt or f32, kind="ExternalInput"
        ).ap()

    q_rows = SHARD if use_collective else SEQ
    # f32r: same bits as f32, but 4x faster PE matmuls (plenty of precision
    # for the 2e-2 tolerance)
    qs = din("qs", (q_rows, H), f32r)
    ks = din("ks", (SHARD, H))
    dg = din("dg", (K, F))
    ts = din("ts", (K,))
    Wt1 = din("Wt1", (1, F // 4))
    bt1 = din("bt1", (F // 4,))
    Wt2 = din("Wt2", (F // 4, F))
    bt2 = din("bt2", (F,))
    Wa1 = din("Wa1", (F + H, F))
    ba1 = din("ba1", (F,))
    Wa2 = din("Wa2", (F, 1))
    ba2 = din("ba2", (1,))
    Wg = din("Wg", (F, H), f32r)
    bg = din("bg", (H,))
    # column of 1/SEQ: the qsum partition-reduce matmul yields the scaled
    # mean contribution directly
    scale_col = din("scale_col", (128, 1), f32r)
    out = nc.dram_tensor("out", [SEQ, SHARD], f16, kind="ExternalOutput").ap()

    def bcast(ap, n):
        # replicate a DRAM row across n partitions (stride-0 partition dim)
        return bass.AP(tensor=ap.tensor, offset=ap.offset, ap=[[0, n]] + list(ap.ap))

    def col(ap, n):
        # load a flat [n] DRAM vector as an [n, 1] column
        return bass.AP(tensor=ap.tensor, offset=ap.offset, ap=[[1, n], [n, 1]])

    with tile.TileContext(nc) as tc:
        with (
            tc.tile_pool(name="consts", bufs=1) as cp,
            tc.tile_pool(name="work", bufs=1) as wp,
            tc.tile_pool(name="qstream", bufs=8) as qp,
            tc.tile_pool(name="scratch", bufs=4) as sp,
            tc.tile_pool(name="psum_small", bufs=1, space="PSUM") as pps,
            tc.tile_pool(name="psum_big", bufs=2, space="PSUM") as ppb,
            tc.tile_pool(name="dram", bufs=1, space="DRAM") as dp,
        ):
            # ---- scale column first (feeds the qsum matmuls) ------------
            sc_c = cp.tile([128, 1], f32r, tag="sc")
            nc.sync.dma_start(sc_c, scale_col)
            # warm the ACT function tables used late in the critical path
            warm1 = cp.tile([1, 1], f32, tag="warm1")
            nc.scalar.activation(warm1, sc_c[0:1, :], AF.Tanh)
            warm2 = cp.tile([1, 1], f32, tag="warm2")
            nc.scalar.activation(warm2, sc_c[0:1, :], AF.Sigmoid)

            # stream order on the (in-order) sync queue:
            #  collective: query -> Wg/bg -> keys (collective path is the
            #    critical chain, so the query shard owns the wire first)
            #  fallback:   keys -> Wg/bg -> query (the 93us replicated query
            #    read dominates; the whole matvec hides under it)
            nq = q_rows // 128
            qv = qs.rearrange("(t p) h -> t p h", p=128)
            kv = ks.rearrange("(p t) h -> p t h", t=NT)

            def load_query():
                tiles, insts = [], []
                for i in range(nq):
                    qt = qp.tile([128, H], f32r, tag="qt")
                    insts.append(nc.sync.dma_start(qt, qv[i]))
                    tiles.append(qt)
                return tiles, insts

            def load_wg():
                Wg_sb = cp.tile([F, H], f32r, tag="Wg")
                nc.sync.dma_start(Wg_sb, Wg)
                bg_sb = cp.tile([1, H], f32, tag="bg")
                nc.sync.dma_start(bg_sb, bg.rearrange("(a h) -> a h", a=1))
                return Wg_sb, bg_sb

            def load_key(j):
                # interleaved: ktiles[j][p, :] = ks[p*NT + j, :]
                kt = cp.tile([128, H], f32, tag=f"ks{j}")
                return kt, nc.sync.dma_start(kt, kv[:, j, :])

            def qsum_block(qtiles):
                # psum[0, h] = sum_s q[s, h] / SEQ — accumulating f32r
                # matmuls, one pair per query tile (keeps DVE free)
                qsum_ps = ppb.tile([1, H], f32, tag="big")
                for i in range(len(qtiles)):
                    nc.tensor.matmul(
                        qsum_ps[:, 0:512], lhsT=sc_c, rhs=qtiles[i][:, 0:512],
                        start=(i == 0), stop=(i == len(qtiles) - 1),
                    )
                    nc.tensor.matmul(
                        qsum_ps[:, 512:1024], lhsT=sc_c,
                        rhs=qtiles[i][:, 512:1024],
                        start=(i == 0), stop=(i == len(qtiles) - 1),
                    )
                qpart_sb = wp.tile([1, H], f32, tag="qpart")
                nc.scalar.copy(qpart_sb, qsum_ps)
                return qpart_sb

            if use_collective:
                qtiles, q_insts = load_query()
                qpart_sb = qsum_block(qtiles)
                Wg_sb, bg_sb = load_wg()
                # reserved wire slot for the 4KB cc_in store before the key
                # stream (on the in-order sync queue it would otherwise be
                # served only after the whole key stream)
                cc_in = dp.tile([1, H], f32, tag="ccin")
                nc.sync.dma_start(cc_in, qpart_sb)
                ktiles, k_insts = [], []
                for j in range(NT):
                    kt, ki = load_key(j)
                    ktiles.append(kt)
                    k_insts.append(ki)
                last_stream = k_insts[-1]
                dep_second = q_insts[1]
            else:
                ktiles, k_insts = [], []
                for j in range(NT):
                    kt, ki = load_key(j)
                    ktiles.append(kt)
                    k_insts.append(ki)
                Wg_sb, bg_sb = load_wg()
                qtiles, q_insts = load_query()
                qpart_sb = qsum_block(qtiles)
                last_stream = q_insts[-1]
                dep_second = k_insts[1]

            # tiny temporal-MLP weights on the ACT queue, deferred behind the
            # second stream tile so their transfers don't interleave into the
            # head of the critical load stream
            Wt2_sb = cp.tile([F // 4, F], f32, tag="Wt2")
            tw = nc.scalar.dma_start(Wt2_sb, Wt2)
            add_dep_helper(tw.ins, dep_second.ins,
                           reason="tiny weights after stream head")
            dgT_sb = cp.tile([F, K], f32, tag="dgT")
            nc.scalar.dma_start(dgT_sb, dg.rearrange("k f -> f k"))
            tb_sb = cp.tile([F // 4, K], f32, tag="tb")
            nc.scalar.dma_start(tb_sb, bcast(ts, F // 4))
            Wt1T_sb = cp.tile([F // 4, 1], f32, tag="Wt1T")
            nc.scalar.dma_start(Wt1T_sb, col(Wt1, F // 4))
            bt1T_sb = cp.tile([F // 4, 1], f32, tag="bt1T")
            nc.scalar.dma_start(bt1T_sb, col(bt1, F // 4))
            bt2T_sb = cp.tile([F, 1], f32, tag="bt2T")
            nc.scalar.dma_start(bt2T_sb, col(bt2, F))

            # ---- scorer weights (needed only post-collective) ----------
            Wa1m_sb = cp.tile([128, 128], f32, tag="Wa1m")
            wb0 = nc.sync.dma_start(Wa1m_sb, Wa1[0:F, :])
            add_dep_helper(wb0.ins, last_stream.ins,
                           reason="scorer weights after the big streams")
            # qmean rows of Wa1 re-paired to the interleaved qmT layout:
            # chunk c pairs with rows {128 + i*8 + c}
            Wa1q_sb = cp.tile([128, 8, 128], f32, tag="Wa1q")
            nc.sync.dma_start(
                Wa1q_sb, Wa1[F : F + H, :].rearrange("(i c) f -> i c f", c=8)
            )
            Wa2_sb = cp.tile([F, 1], f32, tag="Wa2")
            nc.sync.dma_start(Wa2_sb, Wa2)
            ba1T_sb = cp.tile([F, 1], f32, tag="ba1T")
            nc.sync.dma_start(ba1T_sb, col(ba1, F))
            ba2b_sb = cp.tile([1, 1], f32, tag="ba2b")
            nc.sync.dma_start(ba2b_sb, bcast(ba2, 1))

            if use_collective:
                cc_out = dp.tile([NCORES, H], f32, tag="ccout")
                nc.gpsimd.collective_compute(
                    "AllGather",
                    ALU.bypass,
                    replica_groups=[list(range(NCORES))],
                    ins=[cc_in.opt()],
                    outs=[cc_out.opt()],
                )

            # ---- temporal MLP -> memT [F, K] ---------------------------
            h1T = wp.tile([F // 4, K], f32, tag="h1T")
            nc.vector.tensor_scalar_mul(h1T, tb_sb, Wt1T_sb)
            nc.vector.tensor_scalar_add(h1T, h1T, bt1T_sb)
            nc.vector.tensor_relu(h1T, h1T)
            tT_ps = pps.tile([F, K], f32, tag="small")
            nc.tensor.matmul(tT_ps, lhsT=Wt2_sb, rhs=h1T, start=True, stop=True)
            memT_sb = wp.tile([F, K], f32, tag="memT")
            nc.scalar.activation(memT_sb, tT_ps, AF.Identity, bias=bt2T_sb, scale=1.0)
            nc.vector.tensor_add(memT_sb, memT_sb, dgT_sb)
            memTr_sb = wp.tile([F, K], f32r, tag="memTr")
            nc.vector.tensor_copy(memTr_sb, memT_sb)

            # ---- gate rows g_k = mem_k @ Wg + bg  [1, H] ---------------
            def g_row(k):
                g_ps = ppb.tile([1, H], f32, tag="big")
                nc.tensor.matmul(g_ps[:, 0:512], lhsT=memTr_sb[:, k : k + 1],
                                 rhs=Wg_sb[:, 0:512], start=True, stop=True)
                nc.tensor.matmul(g_ps[:, 512:1024], lhsT=memTr_sb[:, k : k + 1],
                                 rhs=Wg_sb[:, 512:1024], start=True, stop=True)
                return g_ps

            g0_ps = g_row(0)
            g0_sb = wp.tile([1, H], f32, tag="g0r")
            nc.vector.tensor_add(g0_sb, g0_ps, bg_sb)
            gb0 = wp.tile([128, H], f32, tag="gb0")
            nc.gpsimd.partition_broadcast(gb0[:, :], g0_sb[:, :])
            g1_ps = g_row(1)
            g1_sb = wp.tile([1, H], f32, tag="g1r")
            nc.vector.tensor_add(g1_sb, g1_ps, bg_sb)
            gb1 = wp.tile([128, H], f32, tag="gb1")
            nc.gpsimd.partition_broadcast(gb1[:, :], g1_sb[:, :])

            # ---- matvec: fused mul+reduce on DVE -----------------------
            # rcc[p, j, k] = sum_h g_k[h] * ks[p*NT+j, h]
            # (NB: tensor_tensor_reduce would fuse mul+reduce on DVE but
            # crashes real TRN2 hardware — keep mul + separate reduce.)
            # Split the 16 muls and 16 reductions across DVE/Pool/ACT so no
            # single engine's serial stream exceeds the key-load window.
            rcc = wp.tile([128, NT, K], f32, tag="rcc")
            pool_mul = {(1, 1), (3, 1), (4, 1), (5, 1), (7, 1)}
            dve_red = {(3, 1), (4, 1), (5, 1), (7, 0), (7, 1)}
            last_dve = last_act = None
            for j in range(NT):
                for k, gb in ((0, gb0), (1, gb1)):
                    if (j, k) in pool_mul:
                        prod = sp.tile([128, H], f32, tag="prodp")
                        nc.gpsimd.tensor_mul(prod, ktiles[j], gb)
                    else:
                        prod = sp.tile([128, H], f32, tag="prodv")
                        last_dve = nc.vector.tensor_mul(prod, ktiles[j], gb)
                    if (j, k) in dve_red:
                        last_dve = nc.vector.tensor_reduce(
                            rcc[:, j, k : k + 1], prod,
                            axis=mybir.AxisListType.X, op=ALU.add,
                        )
                    else:
                        junk = sp.tile([128, H], f32, tag="junk")
                        last_act = nc.scalar.activation(
                            junk, prod, AF.Copy,
                            accum_out=rcc[:, j, k : k + 1],
                        )

            if use_collective:
                # gather-result load parked late on the sync ring (must not
                # block the ACT accum stream behind the collective wait)
                qmTd8 = wp.tile([128, NCORES, 8], f32, tag="qmTd8")
                nc.sync.dma_start(
                    qmTd8, cc_out[:, :].rearrange("d (p c) -> p d c", c=8)
                )


            # ---- post-collective: qmT, scorer, weights -----------------
            # (on Pool — a DVE/ACT placement would park a collective-gated
            # wait in the middle of the in-order matvec streams)
            # qmT[p, c] = qmean[p*8 + c]  (interleaved reshape layout)
            qmT = wp.tile([128, 8], f32, tag="qmT")
            if use_collective:
                # sum gathered partials over d ([p, c, d] view, reduce X);
                # pinned after the matvec so its collective-gated wait cannot
                # stall the in-order DVE stream mid-matvec
                qr = nc.vector.tensor_reduce(
                    qmT, qmTd8[:, :, :].rearrange("p d c -> p c d"),
                    axis=mybir.AxisListType.X, op=ALU.add,
                )
                add_dep_helper(qr.ins, last_dve.ins,
                               reason="qmT reduce after matvec on DVE")
            else:
                nc.scalar.dma_start(qmT, qpart_sb[:, :])
            # mem part of the scorer pre-activation (anchor-dependent)
            haT_ps = pps.tile([F, K], f32, tag="haT")
            nc.tensor.matmul(haT_ps, lhsT=Wa1m_sb, rhs=memT_sb,
                             start=True, stop=True)
            # qmean part is identical for both anchors -> [F, 1], applied as
            # the tanh bias together with ba1 (saves the anchor-duplication)
            hq_ps = pps.tile([F, 1], f32, tag="hq")
            for c in range(8):
                nc.tensor.matmul(hq_ps, lhsT=Wa1q_sb[:, c, :],
                                 rhs=qmT[:, c : c + 1],
                                 start=(c == 0), stop=(c == 7))
            hqb = wp.tile([F, 1], f32, tag="hqb")
            nc.scalar.activation(hqb, hq_ps, AF.Identity, bias=ba1T_sb, scale=1.0)
            aT_sb = wp.tile([F, K], f32, tag="aT")
            th = nc.scalar.activation(aT_sb, haT_ps, AF.Tanh, bias=hqb, scale=1.0)
            add_dep_helper(th.ins, last_act.ins,
                           reason="tanh after matvec accums on ACT")
            scoreT_ps = pps.tile([1, K], f32, tag="small")
            nc.tensor.matmul(scoreT_ps, lhsT=Wa2_sb, rhs=aT_sb, start=True, stop=True)
            wvT_sb = wp.tile([1, K], f32, tag="wvT")
            # (1/K of the anchor mean is folded into Wg/bg host-side)
            nc.scalar.activation(wvT_sb, scoreT_ps, AF.Sigmoid, bias=ba2b_sb, scale=1.0)
            wvb = wp.tile([128, K], f32, tag="wvb")
            nc.gpsimd.partition_broadcast(wvb[:, :], wvT_sb[:, :])

            # ---- combine anchors in the tiny [128, NT] layout ----------
            o_t8 = wp.tile([128, NT], f32, tag="o_t8")
            nc.vector.tensor_scalar_mul(o_t8, rcc[:, :, 1], wvb[:, 1:2])
            o128 = wp.tile([128, NT], f16, tag="o128")
            nc.vector.scalar_tensor_tensor(
                o128, rcc[:, :, 0], wvb[:, 0:1], o_t8, ALU.mult, ALU.add
            )
            # row[0, p*NT + j] = o128[p, j]  (partition-major flatten = s)
            o_row = wp.tile([1, SHARD], f16, tag="o_row")
            nc.sync.dma_start(o_row[:, :], o128[:, :])
            out_sb = wp.tile([128, SHARD], f16, tag="osb")
            nc.gpsimd.partition_broadcast(out_sb[:, :], o_row[:, :])

            # ---- output: 64 x [128 rows, SHARD cols], all rows = row ---
            outv = out.rearrange("(b p) n -> b p n", p=128)
            for b in range(SEQ // 128):
                nc.sync.dma_start(outv[b], out_sb)

    nc.compile()
    return nc


def _get_prog(use_collective: bool):
    key = bool(use_collective)
    if key not in _PROG_CACHE:
        _PROG_CACHE[key] = _build(key)
    return _PROG_CACHE[key]


def _make_in_maps(inputs, use_collective: bool):
    q = np.ascontiguousarray(np.asarray(inputs["query"], np.float32)[0])  # [S,H]
    k = np.ascontiguousarray(np.asarray(inputs["key"], np.float32)[0])  # [S,H]
    common = {
        "dg": np.ascontiguousarray(np.asarray(inputs["dg_features"], np.float32)),
        "ts": np.ascontiguousarray(np.asarray(inputs["timestamps"], np.float32)),
        "Wt1": np.ascontiguousarray(np.asarray(inputs["Wt1"], np.float32)),
        "bt1": np.ascontiguousarray(np.asarray(inputs["bt1"], np.float32)),
        "Wt2": np.ascontiguousarray(np.asarray(inputs["Wt2"], np.float32)),
        "bt2": np.ascontiguousarray(np.asarray(inputs["bt2"], np.float32)),
        "Wa1": np.ascontiguousarray(np.asarray(inputs["Wa1"], np.float32)),
        "ba1": np.ascontiguousarray(np.asarray(inputs["ba1"], np.float32)),
        "Wa2": np.ascontiguousarray(np.asarray(inputs["Wa2"], np.float32)),
        "ba2": np.ascontiguousarray(np.asarray(inputs["ba2"], np.float32)),
        "Wg": np.ascontiguousarray(np.asarray(inputs["Wg"], np.float32) / K),
        "bg": np.ascontiguousarray(np.asarray(inputs["bg"], np.float32) / K),
        "scale_col": np.full((128, 1), 1.0 / 8192.0, np.float32),
    }
    in_maps = []
    for d in range(NCORES):
        m = dict(common)
        m["ks"] = np.ascontiguousarray(k[d * SHARD : (d + 1) * SHARD])
        if use_collective:
            m["qs"] = np.ascontiguousarray(q[d * SHARD : (d + 1) * SHARD])
        else:
            m["qs"] = q
        in_maps.append(m)
    return in_maps


def _run(inputs, use_collective: bool, trace: bool = False):
    from concourse.bass_utils import run_bass_kernel_spmd

    nc = _get_prog(use_collective)
    in_maps = _make_in_maps(inputs, use_collective)
    res = run_bass_kernel_spmd(
        nc, in_maps, core_ids=list(range(NCORES)), trace=trace
    )
    full = np.empty((1, 1, SEQ, SEQ), np.float32)
    for d in range(NCORES):
        full[0, 0, :, d * SHARD : (d + 1) * SHARD] = res.results[d]["out"]
    return full, res


def kernel(**inputs) -> np.ndarray:
    use_collective = os.environ.get("CA1_NO_COLLECTIVE", "0") != "1"
    if use_collective:
        for attempt in range(2):
            try:
                full, _ = _run(inputs, True)
                return full
            except Exception:
                _PROG_CACHE.pop(True, None)
        # fall back to the zero-communication variant (replicated query)
    full, _ = _run(inputs, False)
    return full


# revision 38
# speedup vs baseline: 1.0904x; 1.0094x over previous
"""Trainium2 Bass kernel for nn_CA1AttentionGate.

Computes, for full inputs (B=1, S=8192, H=1024, F=128, K=2):
    temporal = relu(t @ Wt1 + bt1) @ Wt2 + bt2          [K,F]
    mem      = dg_features + temporal                    [K,F]
    qmean    = query.mean(axis=1)                        [1,H]
    score_k  = tanh([mem_k ; qmean] @ Wa1 + ba1) @ Wa2 + ba2
    w_k      = sigmoid(score_k)
    g_k      = mem_k @ Wg + bg                           [K,H]
    row[s]   = (1/K) * sum_k w_k * (g_k . key[s])        [S]
    out      = broadcast(row) -> [1,1,S,S]

Sharding: sequence-parallel over the key/seq axis across 8 cores.  Each
core computes the final gate row for its 1024 key positions and writes
its dense [8192, 1024] column slab of the output.  The slab is written
in fp16 (well within the 2e-2 tolerance; the host upcasts on gather),
halving the dominant output-write traffic.  The only cross-core
quantity is qmean: each core reduces its query shard via PE matmuls
into PSUM and a 4KB AllGather completes the mean (fallback variant
replicates the full query read instead).
"""

import os

import numpy as np

SEQ = 8192
H = 1024
F = 128
K = 2
NCORES = 8
SHARD = SEQ // NCORES  # 1024
NT = SHARD // 128  # 8 key tiles per shard

_PROG_CACHE = {}


def _build(use_collective: bool):
    import concourse.bacc as bacc
    import concourse.bass as bass
    import concourse.tile as tile
    from concourse import mybir
    from concourse.tile_rust import add_dep_helper

    AF = mybir.ActivationFunctionType
    ALU = mybir.AluOpType
    f32 = mybir.dt.float32
    f32r = mybir.dt.float32r
    f16 = mybir.dt.float16

    nc = bacc.Bacc(
        "TRN2",
        target_bir_lowering=False,
        debug=False,
        num_devices=NCORES,
    )

    def din(name, shape, dt=None):
        return nc.dram_tensor(
            name, list(shape), dt or f32, kind="ExternalInput"
        ).ap()

    q_rows = SHARD if use_collective else SEQ
    # f32r: same bits as f32, but 4x faster PE matmuls (plenty of precision
    # for the 2e-2 tolerance)
    qs = din("qs", (q_rows, H), f32r)
    ks = din("ks", (SHARD, H))
    dg = din("dg", (K, F))
    ts = din("ts", (K,))
    Wt1 = din("Wt1", (1, F // 4))
    bt1 = din("bt1", (F // 4,))
    Wt2 = din("Wt2", (F // 4, F))
    bt2 = din("bt2", (F,))
    Wa1 = din("Wa1", (F + H, F))
    ba1 = din("ba1", (F,))
    Wa2 = din("Wa2", (F, 1))
    ba2 = din("ba2", (1,))
    Wg = din("Wg", (F, H), f32r)
    bg = din("bg", (H,))
    # column of 1/SEQ: the qsum partition-reduce matmul yields the scaled
    # mean contribution directly
    scale_col = din("scale_col", (128, 1), f32r)
    out = nc.dram_tensor("out", [SEQ, SHARD], f16, kind="ExternalOutput").ap()

    def bcast(ap, n):
        # replicate a DRAM row across n partitions (stride-0 partition dim)
        return bass.AP(tensor=ap.tensor, offset=ap.offset, ap=[[0, n]] + list(ap.ap))

    def col(ap, n):
        # load a flat [n] DRAM vector as an [n, 1] column
        return bass.AP(tensor=ap.tensor, offset=ap.offset, ap=[[1, n], [n, 1]])

    with tile.TileContext(nc) as tc:
        with (
            tc.tile_pool(name="consts", bufs=1) as cp,
            tc.tile_pool(name="work", bufs=1) as wp,
            tc.tile_pool(name="qstream", bufs=8) as qp,
            tc.tile_pool(name="scratch", bufs=4) as sp,
            tc.tile_pool(name="psum_small", bufs=1, space="PSUM") as pps,
            tc.tile_pool(name="psum_big", bufs=2, space="PSUM") as ppb,
            tc.tile_pool(name="dram", bufs=1, space="DRAM") as dp,
        ):
            # ---- scale column first (feeds the qsum matmuls) ------------
            sc_c = cp.tile([128, 1], f32r, tag="sc")
            nc.sync.dma_start(sc_c, scale_col)
            # warm the ACT function tables used late in the critical path
            warm1 = cp.tile([1, 1], f32, tag="warm1")
            nc.scalar.activation(warm1, sc_c[0:1, :], AF.Tanh)
            warm2 = cp.tile([1, 1], f32, tag="warm2")
            nc.scalar.activation(warm2, sc_c[0:1, :], AF.Sigmoid)

            # stream order on the (in-order) sync queue:
            #  collective: query -> Wg/bg -> keys (collective path is the
            #    critical chain, so the query shard owns the wire first)
            #  fallback:   keys -> Wg/bg -> query (the 93us replicated query
            #    read dominates; the whole matvec hides under it)
            nq = q_rows // 128
            qv = qs.rearrange("(t p) h -> t p h", p=128)
            kv = ks.rearrange("(p t) h -> p t h", t=NT)

            def load_query():
                tiles, insts = [], []
                for i in range(nq):
                    qt = qp.tile([128, H], f32r, tag="qt")
                    insts.append(nc.sync.dma_start(qt, qv[i]))
                    tiles.append(qt)
                return tiles, insts

            def load_wg():
                Wg_sb = cp.tile([F, H], f32r, tag="Wg")
                nc.sync.dma_start(Wg_sb, Wg)
                bg_sb = cp.tile([1, H], f32, tag="bg")
                nc.sync.dma_start(bg_sb, bg.rearrange("(a h) -> a h", a=1))
                return Wg_sb, bg_sb

            def load_key(j):
                # interleaved: ktiles[j][p, :] = ks[p*NT + j, :]
                kt = cp.tile([128, H], f32, tag=f"ks{j}")
                return kt, nc.sync.dma_start(kt, kv[:, j, :])

            def qsum_block(qtiles):
                # psum[0, h] = sum_s q[s, h] / SEQ — accumulating f32r
                # matmuls, one pair per query tile (keeps DVE free)
                qsum_ps = ppb.tile([1, H], f32, tag="big")
                for i in range(len(qtiles)):
                    nc.tensor.matmul(
                        qsum_ps[:, 0:512], lhsT=sc_c, rhs=qtiles[i][:, 0:512],
                        start=(i == 0), stop=(i == len(qtiles) - 1),
                    )
                    nc.tensor.matmul(
                        qsum_ps[:, 512:1024], lhsT=sc_c,
                        rhs=qtiles[i][:, 512:1024],
                        start=(i == 0), stop=(i == len(qtiles) - 1),
                    )
                qpart_sb = wp.tile([1, H], f32, tag="qpart")
                nc.scalar.copy(qpart_sb, qsum_ps)
                return qpart_sb

            if use_collective:
                qtiles, q_insts = load_query()
                qpart_sb = qsum_block(qtiles)
                Wg_sb, bg_sb = load_wg()
                # reserved wire slot for the 4KB cc_in store before the key
                # stream (on the in-order sync queue it would otherwise be
                # served only after the whole key stream)
                cc_in = dp.tile([1, H], f32, tag="ccin")
                nc.sync.dma_start(cc_in, qpart_sb)
                ktiles, k_insts = [], []
                for j in range(NT):
                    kt, ki = load_key(j)
                    ktiles.append(kt)
                    k_insts.append(ki)
                last_stream = k_insts[-1]
                dep_second = q_insts[1]
            else:
                ktiles, k_insts = [], []
                for j in range(NT):
                    kt, ki = load_key(j)
                    ktiles.append(kt)
                    k_insts.append(ki)
                Wg_sb, bg_sb = load_wg()
                qtiles, q_insts = load_query()
                qpart_sb = qsum_block(qtiles)
                last_stream = q_insts[-1]
                dep_second = k_insts[1]

            # tiny temporal-MLP weights on the ACT queue, deferred behind the
            # second stream tile so their transfers don't interleave into the
            # head of the critical load stream
            Wt2_sb = cp.tile([F // 4, F], f32, tag="Wt2")
            tw = nc.scalar.dma_start(Wt2_sb, Wt2)
            add_dep_helper(tw.ins, dep_second.ins,
                           reason="tiny weights after stream head")
            dgT_sb = cp.tile([F, K], f32, tag="dgT")
            nc.scalar.dma_start(dgT_sb, dg.rearrange("k f -> f k"))
            tb_sb = cp.tile([F // 4, K], f32, tag="tb")
            nc.scalar.dma_start(tb_sb, bcast(ts, F // 4))
            Wt1T_sb = cp.tile([F // 4, 1], f32, tag="Wt1T")
            nc.scalar.dma_start(Wt1T_sb, col(Wt1, F // 4))
            bt1T_sb = cp.tile([F // 4, 1], f32, tag="bt1T")
            nc.scalar.dma_start(bt1T_sb, col(bt1, F // 4))
            bt2T_sb = cp.tile([F, 1], f32, tag="bt2T")
            nc.scalar.dma_start(bt2T_sb, col(bt2, F))

            # ---- scorer weights (needed only post-collective) ----------
            Wa1m_sb = cp.tile([128, 128], f32, tag="Wa1m")
            wb0 = nc.sync.dma_start(Wa1m_sb, Wa1[0:F, :])
            add_dep_helper(wb0.ins, last_stream.ins,
                           reason="scorer weights after the big streams")
            # qmean rows of Wa1 re-paired to the interleaved qmT layout:
            # chunk c pairs with rows {128 + i*8 + c}
            Wa1q_sb = cp.tile([128, 8, 128], f32, tag="Wa1q")
            nc.sync.dma_start(
                Wa1q_sb, Wa1[F : F + H, :].rearrange("(i c) f -> i c f", c=8)
            )
            Wa2_sb = cp.tile([F, 1], f32, tag="Wa2")
            nc.sync.dma_start(Wa2_sb, Wa2)
            ba1T_sb = cp.tile([F, 1], f32, tag="ba1T")
            nc.sync.dma_start(ba1T_sb, col(ba1, F))
            ba2b_sb = cp.tile([1, 1], f32, tag="ba2b")
            nc.sync.dma_start(ba2b_sb, bcast(ba2, 1))

            if use_collective:
                cc_out = dp.tile([NCORES, H], f32, tag="ccout")
                nc.gpsimd.collective_compute(
                    "AllGather",
                    ALU.bypass,
                    replica_groups=[list(range(NCORES))],
                    ins=[cc_in.opt()],
                    outs=[cc_out.opt()],
                )

            # ---- temporal MLP -> memT [F, K] ---------------------------
            h1T = wp.tile([F // 4, K], f32, tag="h1T")
            nc.vector.tensor_scalar_mul(h1T, tb_sb, Wt1T_sb)
            nc.vector.tensor_scalar_add(h1T, h1T, bt1T_sb)
            nc.vector.tensor_relu(h1T, h1T)
            tT_ps = pps.tile([F, K], f32, tag="small")
            nc.tensor.matmul(tT_ps, lhsT=Wt2_sb, rhs=h1T, start=True, stop=True)
            memT_sb = wp.tile([F, K], f32, tag="memT")
            nc.scalar.activation(memT_sb, tT_ps, AF.Identity, bias=bt2T_sb, scale=1.0)
            nc.vector.tensor_add(memT_sb, memT_sb, dgT_sb)
            memTr_sb = wp.tile([F, K], f32r, tag="memTr")
            nc.vector.tensor_copy(memTr_sb, memT_sb)

            # ---- gate rows g_k = mem_k @ Wg + bg  [1, H] ---------------
            def g_row(k):
                g_ps = ppb.tile([1, H], f32, tag="big")
                nc.tensor.matmul(g_ps[:, 0:512], lhsT=memTr_sb[:, k : k + 1],
                                 rhs=Wg_sb[:, 0:512], start=True, stop=True)
                nc.tensor.matmul(g_ps[:, 512:1024], lhsT=memTr_sb[:, k : k + 1],
                                 rhs=Wg_sb[:, 512:1024], start=True, stop=True)
                return g_ps

            g0_ps = g_row(0)
            g0_sb = wp.tile([1, H], f32, tag="g0r")
            nc.vector.tensor_add(g0_sb, g0_ps, bg_sb)
            gb0 = wp.tile([128, H], f32, tag="gb0")
            nc.gpsimd.partition_broadcast(gb0[:, :], g0_sb[:, :])
            g1_ps = g_row(1)
            g1_sb = wp.tile([1, H], f32, tag="g1r")
            nc.vector.tensor_add(g1_sb, g1_ps, bg_sb)
            gb1 = wp.tile([128, H], f32, tag="gb1")
            nc.gpsimd.partition_broadcast(gb1[:, :], g1_sb[:, :])

            # ---- matvec: fused mul+reduce on DVE -----------------------
            # rcc[p, j, k] = sum_h g_k[h] * ks[p*NT+j, h]
            # (NB: tensor_tensor_reduce would fuse mul+reduce on DVE but
            # crashes real TRN2 hardware — keep mul + separate reduce.)
            # Split the 16 muls and 16 reductions across DVE/Pool/ACT so no
            # single engine's serial stream exceeds the key-load window.
            rcc = wp.tile([128, NT, K], f32, tag="rcc")
            pool_mul = {(1, 1), (3, 1), (4, 1), (5, 1), (7, 1)}
            dve_red = {(3, 1), (4, 1), (5, 1), (7, 0), (7, 1)}
            last_dve = last_act = None
            for j in range(NT):
                for k, gb in ((0, gb0), (1, gb1)):
                    if (j, k) in pool_mul:
                        prod = sp.tile([128, H], f32, tag="prodp")
                        nc.gpsimd.tensor_mul(prod, ktiles[j], gb)
                    else:
                        prod = sp.tile([128, H], f32, tag="prodv")
                        last_dve = nc.vector.tensor_mul(prod, ktiles[j], gb)
                    if (j, k) in dve_red:
                        last_dve = nc.vector.tensor_reduce(
                            rcc[:, j, k : k + 1], prod,
                            axis=mybir.AxisListType.X, op=ALU.add,
                        )
                    else:
                        junk = sp.tile([128, H], f32, tag="junk")
                        last_act = nc.scalar.activation(
                            junk, prod, AF.Copy,
                            accum_out=rcc[:, j, k : k + 1],
                        )

            if use_collective:
                # gather-result load parked late on the sync ring (must not
                # block the ACT accum stream behind the collective wait)
                qmTd8 = wp.tile([128, NCORES, 8], f32, tag="qmTd8")
                nc.sync.dma_start(
                    qmTd8, cc_out[:, :].rearrange("d (p c) -> p d c", c=8)
                )


            # ---- post-collective: qmT, scorer, weights -----------------
            # (on Pool — a DVE/ACT placement would park a collective-gated
            # wait in the middle of the in-order matvec streams)
            # qmT[p, c] = qmean[p*8 + c]  (interleaved reshape layout)
            qmT = wp.tile([128, 8], f32, tag="qmT")
            if use_collective:
                # sum gathered partials over d ([p, c, d] view, reduce X);
                # pinned after the matvec so its collective-gated wait cannot
                # stall the in-order DVE stream mid-matvec
                qr = nc.vector.tensor_reduce(
                    qmT, qmTd8[:, :, :].rearrange("p d c -> p c d"),
                    axis=mybir.AxisListType.X, op=ALU.add,
                )
                add_dep_helper(qr.ins, last_dve.ins,
                               reason="qmT reduce after matvec on DVE")
            else:
                nc.scalar.dma_start(qmT, qpart_sb[:, :])
            # mem part of the scorer pre-activation (anchor-dependent)
            haT_ps = pps.tile([F, K], f32, tag="haT")
            nc.tensor.matmul(haT_ps, lhsT=Wa1m_sb, rhs=memT_sb,
                             start=True, stop=True)
            # qmean part is identical for both anchors -> [F, 1], applied as
            # the tanh bias together with ba1 (saves the anchor-duplication)
            hq_ps = pps.tile([F, 1], f32, tag="hq")
            for c in range(8):
                nc.tensor.matmul(hq_ps, lhsT=Wa1q_sb[:, c, :],
                                 rhs=qmT[:, c : c + 1],
                                 start=(c == 0), stop=(c == 7))
            hqb = wp.tile([F, 1], f32, tag="hqb")
            nc.scalar.activation(hqb, hq_ps, AF.Identity, bias=ba1T_sb, scale=1.0)
            aT_sb = wp.tile([F, K], f32, tag="aT")
            th = nc.scalar.activation(aT_sb, haT_ps, AF.Tanh, bias=hqb, scale=1.0)
            add_dep_helper(th.ins, last_act.ins,
                           reason="tanh after matvec accums on ACT")
            scoreT_ps = pps.tile([1, K], f32, tag="small")
            nc.tensor.matmul(scoreT_ps, lhsT=Wa2_sb, rhs=aT_sb, start=True, stop=True)
            wvT_sb = wp.tile([1, K], f32, tag="wvT")
            # (1/K of the anchor mean is folded into Wg/bg host-side)
            nc.scalar.activation(wvT_sb, scoreT_ps, AF.Sigmoid, bias=ba2b_sb, scale=1.0)
            wvb = wp.tile([128, K], f32, tag="wvb")
            nc.gpsimd.partition_broadcast(wvb[:, :], wvT_sb[:, :])

            # ---- combine anchors in the tiny [128, NT] layout ----------
            o_t8 = wp.tile([128, NT], f32, tag="o_t8")
            nc.vector.tensor_scalar_mul(o_t8, rcc[:, :, 1], wvb[:, 1:2])
            o128 = wp.tile([128, NT], f16, tag="o128")
            nc.vector.scalar_tensor_tensor(
                o128, rcc[:, :, 0], wvb[:, 0:1], o_t8, ALU.mult, ALU.add
            )
            # row[0, p*NT + j] = o128[p, j]  (partition-major flatten = s)
            o_row = wp.tile([1, SHARD], f16, tag="o_row")
            nc.sync.dma_start(o_row[:, :], o128[:, :])
            # outputs read the single-partition row through a stride-0
            # partition AP (each descriptor re-reads the same 2KB) — no
            # broadcast step between the row and the output stream
            orow_ap = o_row[:, :]
            o_bc = bass.AP(
                tensor=orow_ap.tensor, offset=orow_ap.offset,
                ap=[list(orow_ap.ap[0]), [0, 128]]
                + [list(d) for d in orow_ap.ap[1:]],
            )

            # ---- output: 64 x [128 rows, SHARD cols], all rows = row ---
            outv = out.rearrange("(b p) n -> b p n", p=128)
            for b in range(SEQ // 128):
                nc.sync.dma_start(outv[b], o_bc)

    nc.compile()
    return nc


def _get_prog(use_collective: bool):
    key = bool(use_collective)
    if key not in _PROG_CACHE:
        _PROG_CACHE[key] = _build(key)
    return _PROG_CACHE[key]


def _make_in_maps(inputs, use_collective: bool):
    q = np.ascontiguousarray(np.asarray(inputs["query"], np.float32)[0])  # [S,H]
    k = np.ascontiguousarray(np.asarray(inputs["key"], np.float32)[0])  # [S,H]
    common = {
        "dg": np.ascontiguousarray(np.asarray(inputs["dg_features"], np.float32)),
        "ts": np.ascontiguousarray(np.asarray(inputs["timestamps"], np.float32)),
        "Wt1": np.ascontiguousarray(np.asarray(inputs["Wt1"], np.float32)),
        "bt1": np.ascontiguousarray(np.asarray(inputs["bt1"], np.float32)),
        "Wt2": np.ascontiguousarray(np.asarray(inputs["Wt2"], np.float32)),
        "bt2": np.ascontiguousarray(np.asarray(inputs["bt2"], np.float32)),
        "Wa1": np.ascontiguousarray(np.asarray(inputs["Wa1"], np.float32)),
        "ba1": np.ascontiguousarray(np.asarray(inputs["ba1"], np.float32)),
        "Wa2": np.ascontiguousarray(np.asarray(inputs["Wa2"], np.float32)),
        "ba2": np.ascontiguousarray(np.asarray(inputs["ba2"], np.float32)),
        "Wg": np.ascontiguousarray(np.asarray(inputs["Wg"], np.float32) / K),
        "bg": np.ascontiguousarray(np.asarray(inputs["bg"], np.float32) / K),
        "scale_col": np.full((128, 1), 1.0 / 8192.0, np.float32),
    }
    in_maps = []
    for d in range(NCORES):
        m = dict(common)
        m["ks"] = np.ascontiguousarray(k[d * SHARD : (d + 1) * SHARD])
        if use_collective:
            m["qs"] = np.ascontiguousarray(q[d * SHARD : (d + 1) * SHARD])
        else:
            m["qs"] = q
        in_maps.append(m)
    return in_maps


def _run(inputs, use_collective: bool, trace: bool = False):
    from concourse.bass_utils import run_bass_kernel_spmd

    nc = _get_prog(use_collective)
    in_maps = _make_in_maps(inputs, use_collective)
    res = run_bass_kernel_spmd(
        nc, in_maps, core_ids=list(range(NCORES)), trace=trace
    )
    full = np.empty((1, 1, SEQ, SEQ), np.float32)
    for d in range(NCORES):
        full[0, 0, :, d * SHARD : (d + 1) * SHARD] = res.results[d]["out"]
    return full, res


def kernel(**inputs) -> np.ndarray:
    use_collective = os.environ.get("CA1_NO_COLLECTIVE", "0") != "1"
    if use_collective:
        for attempt in range(2):
            try:
                full, _ = _run(inputs, True)
                return full
            except Exception:
                _PROG_CACHE.pop(True, None)
        # fall back to the zero-communication variant (replicated query)
    full, _ = _run(inputs, False)
    return full


# revision 43
# speedup vs baseline: 1.9699x; 1.8066x over previous
"""Trainium2 Bass kernel for nn_CA1AttentionGate.

Computes, for full inputs (B=1, S=8192, H=1024, F=128, K=2):
    temporal = relu(t @ Wt1 + bt1) @ Wt2 + bt2          [K,F]
    mem      = dg_features + temporal                    [K,F]
    qmean    = query.mean(axis=1)                        [1,H]
    score_k  = tanh([mem_k ; qmean] @ Wa1 + ba1) @ Wa2 + ba2
    w_k      = sigmoid(score_k)
    g_k      = mem_k @ Wg + bg                           [K,H]
    row[s]   = (1/K) * sum_k w_k * (g_k . key[s])        [S]
    out      = broadcast(row) -> [1,1,S,S]

Sharding: sequence-parallel over the key/seq axis across 8 cores.  Each
core computes the final gate row for its 1024 key positions and writes
its dense [8192, 1024] column slab of the output.  The slab is written
in fp16 (well within the 2e-2 tolerance; the host upcasts on gather),
halving the dominant output-write traffic.  The only cross-core
quantity is qmean: each core reduces its query shard via PE matmuls
into PSUM and a 4KB AllGather completes the mean (fallback variant
replicates the full query read instead).
"""

import os

import numpy as np

SEQ = 8192
H = 1024
F = 128
K = 2
NCORES = 8
SHARD = SEQ // NCORES  # 1024
NT = SHARD // 128  # 8 key tiles per shard

_PROG_CACHE = {}


def _build(use_collective: bool):
    import concourse.bacc as bacc
    import concourse.bass as bass
    import concourse.tile as tile
    from concourse import mybir
    from concourse.tile_rust import add_dep_helper

    AF = mybir.ActivationFunctionType
    ALU = mybir.AluOpType
    f32 = mybir.dt.float32
    f32r = mybir.dt.float32r
    f16 = mybir.dt.float16

    nc = bacc.Bacc(
        "TRN2",
        target_bir_lowering=False,
        debug=False,
        num_devices=NCORES,
    )

    def din(name, shape, dt=None):
        return nc.dram_tensor(
            name, list(shape), dt or f32, kind="ExternalInput"
        ).ap()

    q_rows = SHARD if use_collective else SEQ
    # f32r: same bits as f32, but 4x faster PE matmuls (plenty of precision
    # for the 2e-2 tolerance)
    qs = din("qs", (q_rows, H), f32r)
    ks = din("ks", (SHARD, H))
    dg = din("dg", (K, F))
    ts = din("ts", (K,))
    Wt1 = din("Wt1", (1, F // 4))
    bt1 = din("bt1", (F // 4,))
    Wt2 = din("Wt2", (F // 4, F))
    bt2 = din("bt2", (F,))
    Wa1 = din("Wa1", (F + H, F))
    ba1 = din("ba1", (F,))
    Wa2 = din("Wa2", (F, 1))
    ba2 = din("ba2", (1,))
    Wg = din("Wg", (F, H), f32r)
    bg = din("bg", (H,))
    # column of 1/SEQ: the qsum partition-reduce matmul yields the scaled
    # mean contribution directly
    scale_col = din("scale_col", (128, 1), f32r)
    out = nc.dram_tensor("out", [SEQ, SHARD], f16, kind="ExternalOutput").ap()

    def bcast(ap, n):
        # replicate a DRAM row across n partitions (stride-0 partition dim)
        return bass.AP(tensor=ap.tensor, offset=ap.offset, ap=[[0, n]] + list(ap.ap))

    def col(ap, n):
        # load a flat [n] DRAM vector as an [n, 1] column
        return bass.AP(tensor=ap.tensor, offset=ap.offset, ap=[[1, n], [n, 1]])

    with tile.TileContext(nc) as tc:
        with (
            tc.tile_pool(name="consts", bufs=1) as cp,
            tc.tile_pool(name="work", bufs=1) as wp,
            tc.tile_pool(name="qstream", bufs=8) as qp,
            tc.tile_pool(name="scratch", bufs=4) as sp,
            tc.tile_pool(name="psum_small", bufs=1, space="PSUM") as pps,
            tc.tile_pool(name="psum_big", bufs=2, space="PSUM") as ppb,
            tc.tile_pool(name="dram", bufs=1, space="DRAM") as dp,
        ):
            # ---- scale column first (feeds the qsum matmuls) ------------
            sc_c = cp.tile([128, 1], f32r, tag="sc")
            nc.sync.dma_start(sc_c, scale_col)
            # warm the ACT function tables used late in the critical path
            warm1 = cp.tile([1, 1], f32, tag="warm1")
            nc.scalar.activation(warm1, sc_c[0:1, :], AF.Tanh)
            warm2 = cp.tile([1, 1], f32, tag="warm2")
            nc.scalar.activation(warm2, sc_c[0:1, :], AF.Sigmoid)

            # stream order on the (in-order) sync queue:
            #  collective: query -> Wg/bg -> keys (collective path is the
            #    critical chain, so the query shard owns the wire first)
            #  fallback:   keys -> Wg/bg -> query (the 93us replicated query
            #    read dominates; the whole matvec hides under it)
            nq = q_rows // 128
            qv = qs.rearrange("(t p) h -> t p h", p=128)
            kv = ks.rearrange("(p t) h -> p t h", t=NT)

            def load_query():
                tiles, insts = [], []
                for i in range(nq):
                    qt = qp.tile([128, H], f32r, tag="qt")
                    insts.append(nc.sync.dma_start(qt, qv[i]))
                    tiles.append(qt)
                return tiles, insts

            def load_wg():
                Wg_sb = cp.tile([F, H], f32r, tag="Wg")
                nc.sync.dma_start(Wg_sb, Wg)
                bg_sb = cp.tile([1, H], f32, tag="bg")
                nc.sync.dma_start(bg_sb, bg.rearrange("(a h) -> a h", a=1))
                return Wg_sb, bg_sb

            def load_key(j):
                # interleaved: ktiles[j][p, :] = ks[p*NT + j, :]
                kt = cp.tile([128, H], f32, tag=f"ks{j}")
                return kt, nc.sync.dma_start(kt, kv[:, j, :])

            def qsum_block(qtiles):
                # psum[0, h] = sum_s q[s, h] / SEQ — accumulating f32r
                # matmuls, one pair per query tile (keeps DVE free)
                qsum_ps = ppb.tile([1, H], f32, tag="big")
                for i in range(len(qtiles)):
                    nc.tensor.matmul(
                        qsum_ps[:, 0:512], lhsT=sc_c, rhs=qtiles[i][:, 0:512],
                        start=(i == 0), stop=(i == len(qtiles) - 1),
                    )
                    nc.tensor.matmul(
                        qsum_ps[:, 512:1024], lhsT=sc_c,
                        rhs=qtiles[i][:, 512:1024],
                        start=(i == 0), stop=(i == len(qtiles) - 1),
                    )
                qpart_sb = wp.tile([1, H], f32, tag="qpart")
                nc.scalar.copy(qpart_sb, qsum_ps)
                return qpart_sb

            if use_collective:
                qtiles, q_insts = load_query()
                qpart_sb = qsum_block(qtiles)
                Wg_sb, bg_sb = load_wg()
                # reserved wire slot for the 4KB cc_in store before the key
                # stream (on the in-order sync queue it would otherwise be
                # served only after the whole key stream)
                cc_in = dp.tile([1, H], f32, tag="ccin")
                nc.sync.dma_start(cc_in, qpart_sb)
                ktiles, k_insts = [], []
                for j in range(NT):
                    kt, ki = load_key(j)
                    ktiles.append(kt)
                    k_insts.append(ki)
                last_stream = k_insts[-1]
                dep_second = q_insts[1]
            else:
                ktiles, k_insts = [], []
                for j in range(NT):
                    kt, ki = load_key(j)
                    ktiles.append(kt)
                    k_insts.append(ki)
                Wg_sb, bg_sb = load_wg()
                qtiles, q_insts = load_query()
                qpart_sb = qsum_block(qtiles)
                last_stream = q_insts[-1]
                dep_second = k_insts[1]

            # tiny temporal-MLP weights on the ACT queue, deferred behind the
            # second stream tile so their transfers don't interleave into the
            # head of the critical load stream
            Wt2_sb = cp.tile([F // 4, F], f32, tag="Wt2")
            tw = nc.scalar.dma_start(Wt2_sb, Wt2)
            add_dep_helper(tw.ins, dep_second.ins,
                           reason="tiny weights after stream head")
            dgT_sb = cp.tile([F, K], f32, tag="dgT")
            nc.scalar.dma_start(dgT_sb, dg.rearrange("k f -> f k"))
            tb_sb = cp.tile([F // 4, K], f32, tag="tb")
            nc.scalar.dma_start(tb_sb, bcast(ts, F // 4))
            Wt1T_sb = cp.tile([F // 4, 1], f32, tag="Wt1T")
            nc.scalar.dma_start(Wt1T_sb, col(Wt1, F // 4))
            bt1T_sb = cp.tile([F // 4, 1], f32, tag="bt1T")
            nc.scalar.dma_start(bt1T_sb, col(bt1, F // 4))
            bt2T_sb = cp.tile([F, 1], f32, tag="bt2T")
            nc.scalar.dma_start(bt2T_sb, col(bt2, F))

            # ---- scorer weights (needed only post-collective) ----------
            Wa1m_sb = cp.tile([128, 128], f32, tag="Wa1m")
            wb0 = nc.sync.dma_start(Wa1m_sb, Wa1[0:F, :])
            add_dep_helper(wb0.ins, last_stream.ins,
                           reason="scorer weights after the big streams")
            # qmean rows of Wa1 re-paired to the interleaved qmT layout:
            # chunk c pairs with rows {128 + i*8 + c}
            Wa1q_sb = cp.tile([128, 8, 128], f32, tag="Wa1q")
            nc.sync.dma_start(
                Wa1q_sb, Wa1[F : F + H, :].rearrange("(i c) f -> i c f", c=8)
            )
            Wa2_sb = cp.tile([F, 1], f32, tag="Wa2")
            nc.sync.dma_start(Wa2_sb, Wa2)
            ba1T_sb = cp.tile([F, 1], f32, tag="ba1T")
            nc.sync.dma_start(ba1T_sb, col(ba1, F))
            ba2b_sb = cp.tile([1, 1], f32, tag="ba2b")
            nc.sync.dma_start(ba2b_sb, bcast(ba2, 1))

            if use_collective:
                cc_out = dp.tile([NCORES, H], f32, tag="ccout")
                nc.gpsimd.collective_compute(
                    "AllGather",
                    ALU.bypass,
                    replica_groups=[list(range(NCORES))],
                    ins=[cc_in.opt()],
                    outs=[cc_out.opt()],
                )

            # ---- temporal MLP -> memT [F, K] ---------------------------
            h1T = wp.tile([F // 4, K], f32, tag="h1T")
            nc.vector.tensor_scalar_mul(h1T, tb_sb, Wt1T_sb)
            nc.vector.tensor_scalar_add(h1T, h1T, bt1T_sb)
            nc.vector.tensor_relu(h1T, h1T)
            tT_ps = pps.tile([F, K], f32, tag="small")
            nc.tensor.matmul(tT_ps, lhsT=Wt2_sb, rhs=h1T, start=True, stop=True)
            memT_sb = wp.tile([F, K], f32, tag="memT")
            nc.scalar.activation(memT_sb, tT_ps, AF.Identity, bias=bt2T_sb, scale=1.0)
            nc.vector.tensor_add(memT_sb, memT_sb, dgT_sb)
            memTr_sb = wp.tile([F, K], f32r, tag="memTr")
            nc.vector.tensor_copy(memTr_sb, memT_sb)

            # ---- gate rows g_k = mem_k @ Wg + bg  [1, H] ---------------
            def g_row(k):
                g_ps = ppb.tile([1, H], f32, tag="big")
                nc.tensor.matmul(g_ps[:, 0:512], lhsT=memTr_sb[:, k : k + 1],
                                 rhs=Wg_sb[:, 0:512], start=True, stop=True)
                nc.tensor.matmul(g_ps[:, 512:1024], lhsT=memTr_sb[:, k : k + 1],
                                 rhs=Wg_sb[:, 512:1024], start=True, stop=True)
                return g_ps

            g0_ps = g_row(0)
            g0_sb = wp.tile([1, H], f32, tag="g0r")
            nc.vector.tensor_add(g0_sb, g0_ps, bg_sb)
            gb0 = wp.tile([128, H], f32, tag="gb0")
            nc.gpsimd.partition_broadcast(gb0[:, :], g0_sb[:, :])
            g1_ps = g_row(1)
            g1_sb = wp.tile([1, H], f32, tag="g1r")
            nc.vector.tensor_add(g1_sb, g1_ps, bg_sb)
            gb1 = wp.tile([128, H], f32, tag="gb1")
            nc.gpsimd.partition_broadcast(gb1[:, :], g1_sb[:, :])

            # ---- matvec: fused mul+reduce on DVE -----------------------
            # rcc[p, j, k] = sum_h g_k[h] * ks[p*NT+j, h]
            # (NB: tensor_tensor_reduce would fuse mul+reduce on DVE but
            # crashes real TRN2 hardware — keep mul + separate reduce.)
            # Split the 16 muls and 16 reductions across DVE/Pool/ACT so no
            # single engine's serial stream exceeds the key-load window.
            rcc = wp.tile([128, NT, K], f32, tag="rcc")
            pool_mul = {(1, 1), (3, 1), (4, 1), (5, 1), (7, 1)}
            dve_red = {(3, 1), (4, 1), (5, 1), (7, 0), (7, 1)}
            last_dve = last_act = None
            for j in range(NT):
                for k, gb in ((0, gb0), (1, gb1)):
                    if (j, k) in pool_mul:
                        prod = sp.tile([128, H], f32, tag="prodp")
                        nc.gpsimd.tensor_mul(prod, ktiles[j], gb)
                    else:
                        prod = sp.tile([128, H], f32, tag="prodv")
                        last_dve = nc.vector.tensor_mul(prod, ktiles[j], gb)
                    if (j, k) in dve_red:
                        last_dve = nc.vector.tensor_reduce(
                            rcc[:, j, k : k + 1], prod,
                            axis=mybir.AxisListType.X, op=ALU.add,
                        )
                    else:
                        junk = sp.tile([128, H], f32, tag="junk")
                        last_act = nc.scalar.activation(
                            junk, prod, AF.Copy,
                            accum_out=rcc[:, j, k : k + 1],
                        )

            if use_collective:
                # gather-result load parked late on the sync ring (must not
                # block the ACT accum stream behind the collective wait)
                qmTd8 = wp.tile([128, NCORES, 8], f32, tag="qmTd8")
                nc.sync.dma_start(
                    qmTd8, cc_out[:, :].rearrange("d (p c) -> p d c", c=8)
                )


            # ---- post-collective: qmT, scorer, weights -----------------
            # (on Pool — a DVE/ACT placement would park a collective-gated
            # wait in the middle of the in-order matvec streams)
            # qmT[p, c] = qmean[p*8 + c]  (interleaved reshape layout)
            qmT = wp.tile([128, 8], f32, tag="qmT")
            if use_collective:
                # sum gathered partials over d ([p, c, d] view, reduce X);
                # pinned after the matvec so its collective-gated wait cannot
                # stall the in-order DVE stream mid-matvec
                qr = nc.vector.tensor_reduce(
                    qmT, qmTd8[:, :, :].rearrange("p d c -> p c d"),
                    axis=mybir.AxisListType.X, op=ALU.add,
                )
                add_dep_helper(qr.ins, last_dve.ins,
                               reason="qmT reduce after matvec on DVE")
            else:
                nc.scalar.dma_start(qmT, qpart_sb[:, :])
            # mem part of the scorer pre-activation (anchor-dependent)
            haT_ps = pps.tile([F, K], f32, tag="haT")
            nc.tensor.matmul(haT_ps, lhsT=Wa1m_sb, rhs=memT_sb,
                             start=True, stop=True)
            # qmean part is identical for both anchors -> [F, 1], applied as
            # the tanh bias together with ba1 (saves the anchor-duplication)
            hq_ps = pps.tile([F, 1], f32, tag="hq")
            for c in range(8):
                nc.tensor.matmul(hq_ps, lhsT=Wa1q_sb[:, c, :],
                                 rhs=qmT[:, c : c + 1],
                                 start=(c == 0), stop=(c == 7))
            hqb = wp.tile([F, 1], f32, tag="hqb")
            nc.scalar.activation(hqb, hq_ps, AF.Identity, bias=ba1T_sb, scale=1.0)
            aT_sb = wp.tile([F, K], f32, tag="aT")
            th = nc.scalar.activation(aT_sb, haT_ps, AF.Tanh, bias=hqb, scale=1.0)
            add_dep_helper(th.ins, last_act.ins,
                           reason="tanh after matvec accums on ACT")
            scoreT_ps = pps.tile([1, K], f32, tag="small")
            nc.tensor.matmul(scoreT_ps, lhsT=Wa2_sb, rhs=aT_sb, start=True, stop=True)
            wvT_sb = wp.tile([1, K], f32, tag="wvT")
            # (1/K of the anchor mean is folded into Wg/bg host-side)
            nc.scalar.activation(wvT_sb, scoreT_ps, AF.Sigmoid, bias=ba2b_sb, scale=1.0)
            wvb = wp.tile([128, K], f32, tag="wvb")
            nc.gpsimd.partition_broadcast(wvb[:, :], wvT_sb[:, :])

            # ---- combine anchors in the tiny [128, NT] layout ----------
            o_t8 = wp.tile([128, NT], f32, tag="o_t8")
            nc.vector.tensor_scalar_mul(o_t8, rcc[:, :, 1], wvb[:, 1:2])
            o128 = wp.tile([128, NT], f16, tag="o128")
            nc.vector.scalar_tensor_tensor(
                o128, rcc[:, :, 0], wvb[:, 0:1], o_t8, ALU.mult, ALU.add
            )
            # row[0, p*NT + j] = o128[p, j]  (partition-major flatten = s)
            o_row = wp.tile([1, SHARD], f16, tag="o_row")
            nc.sync.dma_start(o_row[:, :], o128[:, :])
            # outputs read the single-partition row through a stride-0
            # partition AP (each descriptor re-reads the same 2KB) — no
            # broadcast step between the row and the output stream
            orow_ap = o_row[:, :]
            o_bc = bass.AP(
                tensor=orow_ap.tensor, offset=orow_ap.offset,
                ap=[list(orow_ap.ap[0]), [0, 128]]
                + [list(d) for d in orow_ap.ap[1:]],
            )

            # ---- output: 64 x [128 rows, SHARD cols], all rows = row ---
            outv = out.rearrange("(b p) n -> b p n", p=128)
            for b in range(SEQ // 128):
                nc.sync.dma_start(outv[b], o_bc)

    nc.compile()
    return nc


def _get_prog(use_collective: bool):
    key = bool(use_collective)
    if key not in _PROG_CACHE:
        _PROG_CACHE[key] = _build(key)
    return _PROG_CACHE[key]


def _make_in_maps(inputs, use_collective: bool):
    q = np.ascontiguousarray(np.asarray(inputs["query"], np.float32)[0])  # [S,H]
    k = np.ascontiguousarray(np.asarray(inputs["key"], np.float32)[0])  # [S,H]
    common = {
        "dg": np.ascontiguousarray(np.asarray(inputs["dg_features"], np.float32)),
        "ts": np.ascontiguousarray(np.asarray(inputs["timestamps"], np.float32)),
        "Wt1": np.ascontiguousarray(np.asarray(inputs["Wt1"], np.float32)),
        "bt1": np.ascontiguousarray(np.asarray(inputs["bt1"], np.float32)),
        "Wt2": np.ascontiguousarray(np.asarray(inputs["Wt2"], np.float32)),
        "bt2": np.ascontiguousarray(np.asarray(inputs["bt2"], np.float32)),
        "Wa1": np.ascontiguousarray(np.asarray(inputs["Wa1"], np.float32)),
        "ba1": np.ascontiguousarray(np.asarray(inputs["ba1"], np.float32)),
        "Wa2": np.ascontiguousarray(np.asarray(inputs["Wa2"], np.float32)),
        "ba2": np.ascontiguousarray(np.asarray(inputs["ba2"], np.float32)),
        "Wg": np.ascontiguousarray(np.asarray(inputs["Wg"], np.float32) / K),
        "bg": np.ascontiguousarray(np.asarray(inputs["bg"], np.float32) / K),
        "scale_col": np.full((128, 1), 1.0 / 8192.0, np.float32),
    }
    in_maps = []
    for d in range(NCORES):
        m = dict(common)
        m["ks"] = np.ascontiguousarray(k[d * SHARD : (d + 1) * SHARD])
        if use_collective:
            m["qs"] = np.ascontiguousarray(q[d * SHARD : (d + 1) * SHARD])
        else:
            m["qs"] = q
        in_maps.append(m)
    return in_maps


def _run(inputs, use_collective: bool, trace: bool = False):
    from concourse.bass_utils import run_bass_kernel_spmd

    nc = _get_prog(use_collective)
    in_maps = _make_in_maps(inputs, use_collective)
    res = run_bass_kernel_spmd(
        nc, in_maps, core_ids=list(range(NCORES)), trace=trace
    )
    full = np.empty((1, 1, SEQ, SEQ), np.float32)
    for d in range(NCORES):
        full[0, 0, :, d * SHARD : (d + 1) * SHARD] = res.results[d]["out"]
    return full, res


def kernel(**inputs) -> np.ndarray:
    use_collective = os.environ.get("CA1_NO_COLLECTIVE", "0") != "1"
    if use_collective:
        for attempt in range(2):
            try:
                full, _ = _run(inputs, True)
                return full
            except Exception:
                _PROG_CACHE.pop(True, None)
        # fall back to the zero-communication variant (replicated query)
    full, _ = _run(inputs, False)
    return full
